# revision 9
# baseline (speedup 1.0000x reference)
"""Trainium2 Bass kernel for nn_MultiHeadAttention (B=2, S=2048, D=1024, H=16).

Sharding: 8 cores = 2 batch groups x 4 cores. Core c handles batch c//4 and
heads 4*(c%4) .. 4*(c%4)+4 (two head-pairs p=0,1). Each core computes Q/K/V
projections for its batch+heads, transposed-layout attention, and a partial
output projection over its 256 head-dims. Host sums the 4 partials per batch.

v3 design (vs v2 baseline @211.7us):
- ACT exp is the roofline (~143us busy): the schedule aims to start the exp
  stream ASAP and never starve it.
- 3 parallel DMA queues (sync/scalar HWDGE + gpsimd SWDGE); the ramp's
  critical Q/K loads split across queues (xq0 in two half-DMAs).
- all non-score PE work (projections, ctx, den matmuls, out-proj) is sliced
  into <~1us items in a FIFO insert queue, popped between score-chunk pairs
  with a per-point cost budget: the 2-deep sc PSUM ring only buffers 2.2us
  of ACT runway, so no insert slab may exceed it.
- den/bc broadcast matmuls in fp16 (were fp32 LOW_HIGH, 3x slower); the
  den->recip->bc chain is split across points so PE never waits on DVE.
- den tree adds C,D on the otherwise-idle gpsimd (DVE was co-bottleneck).
- tail: den pre-summed to one post-exp add, evictions split DVE/ACT,
  out-DMA split over both HWDGE queues, PE kept warm.
"""

import numpy as np

B, S, D, H = 2, 2048, 1024, 16
HD = D // H          # 64
NCORES = 8
HPC = 4              # heads per core
CHD = HPC * HD       # 256 head-dims per core
TOK = S              # tokens per core (one batch)
QW = 512             # query window
NQW = TOK // QW      # 4 windows
NKT = TOK // 128     # 16 key tiles
SCALE = 1.0 / np.sqrt(np.float32(D))  # 1/32, folded into Wq on host

_PROG = None
_LAST_IN_MAPS = None


def _build():
    from contextlib import ExitStack

    import concourse.bass as bass
    import concourse.tile as tile
    from concourse import bacc, mybir

    F16 = mybir.dt.float16
    F32 = mybir.dt.float32
    EXP = mybir.ActivationFunctionType.Exp

    nc = bacc.Bacc("TRN2", target_bir_lowering=False, debug=False,
                   num_devices=NCORES)

    xqT = nc.dram_tensor("xqT", [D, TOK], F16, kind="ExternalInput").ap()
    xkT = nc.dram_tensor("xkT", [D, TOK], F16, kind="ExternalInput").ap()
    xvT = nc.dram_tensor("xvT", [D, TOK], F16, kind="ExternalInput").ap()
    wqT = nc.dram_tensor("wqT", [D, CHD], F16, kind="ExternalInput").ap()
    wkT = nc.dram_tensor("wkT", [D, CHD], F16, kind="ExternalInput").ap()
    wvT = nc.dram_tensor("wvT", [D, CHD], F16, kind="ExternalInput").ap()
    woTs = nc.dram_tensor("woTs", [CHD, D], F16, kind="ExternalInput").ap()
    pout = nc.dram_tensor("pout", [TOK, D], F16, kind="ExternalOutput").ap()

    CB = 512            # V column-block width (tokens)

    with tile.TileContext(nc) as tc, ExitStack() as ctx:
        const = ctx.enter_context(tc.tile_pool(name="const", bufs=1))
        wq_sb = const.tile([128, 8, CHD], F16, tag="wq")
        wk_sb = const.tile([128, 8, CHD], F16, tag="wk")
        wv_sb = const.tile([128, 8, CHD], F16, tag="wv")
        wo_sb = [const.tile([128, D], F16, tag=f"wo{p}", name=f"wo{p}")
                 for p in range(2)]
        onesK = const.tile([128, 1], F16, tag="onesK")
        ones1 = const.tile([1, 128], F32, tag="ones1")

        nc.vector.memset(onesK, 1.0)
        nc.vector.memset(ones1, 1.0)

        warm = const.tile([1, 8], F32, tag="warm")
        nc.vector.memset(warm, 0.0)
        nc.scalar.activation(out=warm, in_=warm, func=EXP)
        wmt = const.tile([128, 64], F16, tag="wmt")
        nc.vector.memset(wmt, 1.0)

        big = ctx.enter_context(tc.tile_pool(name="big", bufs=1))
        KT = big.tile([128, 2, TOK], F16, tag="kt")          # [hd, pair, keys]
        vnat = big.tile([128, NKT, CHD], F16, tag="vnat")    # [keys, kt, hd]
        ctxP = [big.tile([128, TOK], F16, tag=f"ctxP{p}", name=f"ctxP{p}")
                for p in range(2)]

        xkb = ctx.enter_context(tc.tile_pool(name="xkb", bufs=3))
        xkb2 = ctx.enter_context(tc.tile_pool(name="xkb2", bufs=2))
        xvb = ctx.enter_context(tc.tile_pool(name="xvb", bufs=3))
        xqb = ctx.enter_context(tc.tile_pool(name="xqb", bufs=2))
        qtp = ctx.enter_context(tc.tile_pool(name="qtp", bufs=2))
        ptp = ctx.enter_context(tc.tile_pool(name="ptp", bufs=2))
        dtp = ctx.enter_context(tc.tile_pool(name="dtp", bufs=2))
        rrp = ctx.enter_context(tc.tile_pool(name="rrp", bufs=2))
        oev = ctx.enter_context(tc.tile_pool(name="oev", bufs=4))

        # PSUM (8 banks): sc ring 2x[128,1024] (4) + cop ring 2x[128,512] (2)
        # + shared proj/out/den/bcast pool 2x[128,512] (2)
        scp = ctx.enter_context(tc.tile_pool(name="scp", bufs=2, space="PSUM"))
        copp = ctx.enter_context(tc.tile_pool(name="copp", bufs=2, space="PSUM"))
        pp = ctx.enter_context(tc.tile_pool(name="pp", bufs=2, space="PSUM"))

        st = {}  # cross-item state (psum tiles, tree accumulators, rrc)

        # ---------- emission helpers ----------
        def dma_x_block(pool, src, c0, w, name, queue=None, halves=False):
            t = pool.tile([128, 8, w], F16, tag="xb", name=name)
            r = src[:, c0:c0 + w].rearrange("(ko ki) t -> ki ko t", ki=128)
            if halves:
                nc.sync.dma_start(out=t[:, 0:4, :], in_=r[:, 0:4, :])
                nc.scalar.dma_start(out=t[:, 4:8, :], in_=r[:, 4:8, :])
            else:
                (queue or nc.sync).dma_start(out=t, in_=r)
            return t

        def proj_q_quarter(qt, m, xq_t, phase, name):
            if phase == 0:
                st[name] = pp.tile([128, QW], F32, tag="pp", name=name)
            psq = st[name]
            for ko in range(4 * phase, 4 * phase + 4):
                nc.tensor.matmul(
                    psq[:], wq_sb[:, ko, m * 128:(m + 1) * 128],
                    xq_t[:, ko, :], start=(ko == 0), stop=(ko == 7))
            if phase == 1:
                nc.vector.tensor_copy(qt[:, m, :], psq[:])
                del st[name]

        def proj_k_half(c0, w, m, xk_t):
            psk = pp.tile([128, w], F32, tag="pp", name=f"psk{c0}_{m}")
            for ko in range(8):
                nc.tensor.matmul(
                    psk[:], wk_sb[:, ko, m * 128:(m + 1) * 128],
                    xk_t[:, ko, :], start=(ko == 0), stop=(ko == 7))
            nc.vector.tensor_copy(KT[:, m, c0:c0 + w], psk[:])

        def proj_v_slice(blk, half, xv_t, tt):
            name = f"pv{blk}_{half}"
            if tt == 0:
                st[name] = pp.tile([128, 2 * CHD], F32, tag="pp", name=name)
            pv = st[name]
            t0 = (2 * half + tt) * 128
            for ko in range(8):
                nc.tensor.matmul(
                    pv[:, tt * CHD:(tt + 1) * CHD],
                    xv_t[:, ko, t0:t0 + 128],
                    wv_sb[:, ko, :], start=(ko == 0), stop=(ko == 7))
            if tt == 1:
                nc.vector.tensor_copy(
                    vnat[:, 4 * blk + 2 * half:4 * blk + 2 * half + 2, :],
                    bass.AP(tensor=pv.tensor, offset=pv.offset,
                            ap=[list(pv.ap[0]), [CHD, 2], [1, CHD]]))
                del st[name]

        def chunk(p, kt_i, qt, ptblk, tag=""):
            k0 = kt_i * 128
            sc = scp.tile([128, 2 * QW], F32, tag="sc",
                          name=f"sc{tag}_{kt_i}")
            nc.tensor.matmul(
                sc[:, 0:QW], KT[0:64, p, k0:k0 + 128], qt[0:64, p, :],
                start=True, stop=True, tile_position=(0, 0))
            nc.tensor.matmul(
                sc[:, QW:2 * QW], KT[64:128, p, k0:k0 + 128], qt[64:128, p, :],
                start=True, stop=True, tile_position=(64, 0))
            nc.scalar.activation(out=ptblk[:, kt_i, :], in_=sc[:], func=EXP)

        def emit_ctx(p, kt_i, ptblk, cop):
            h0 = p * 128
            nc.tensor.matmul(
                cop[0:64, :], vnat[:, kt_i, h0:h0 + 64],
                ptblk[:, kt_i, 0:QW],
                start=(kt_i == 0), stop=(kt_i == NKT - 1),
                tile_position=(0, 0))
            nc.tensor.matmul(
                cop[64:128, :], vnat[:, kt_i, h0 + 64:h0 + 128],
                ptblk[:, kt_i, QW:2 * QW],
                start=(kt_i == 0), stop=(kt_i == NKT - 1),
                tile_position=(0, 64))

        def emit_ctx_duo(p, d, ptblk, cop):
            emit_ctx(p, 2 * d, ptblk, cop)
            emit_ctx(p, 2 * d + 1, ptblk, cop)

        def tree_lo(ptblk, key):
            stt = dtp.tile([128, 2, 2 * QW], F16, tag="dt", name=f"dt_{key}")
            pv = lambda a, b: ptblk[:, a:b, :]
            nc.vector.tensor_add(stt[:, 0:2, :], pv(0, 2), pv(2, 4))    # A
            nc.vector.tensor_add(pv(0, 2), pv(4, 6), pv(6, 8))          # B
            nc.vector.tensor_add(pv(4, 6), stt[:, 0:2, :], pv(0, 2))    # E
            st[key] = stt

        def tree_hi(ptblk, key):
            pv = lambda a, b: ptblk[:, a:b, :]
            nc.gpsimd.tensor_add(pv(2, 4), pv(8, 10), pv(10, 12))       # C
            nc.gpsimd.tensor_add(pv(6, 8), pv(12, 14), pv(14, 16))      # D
            nc.vector.tensor_add(pv(8, 10), pv(2, 4), pv(6, 8))         # F
            nc.vector.tensor_add(pv(0, 2), pv(4, 6), pv(8, 10))         # G
            nc.vector.tensor_add(ptblk[:, 2, :], ptblk[:, 0, :],
                                 ptblk[:, 1, :])                        # acc

        def den_mm(ptblk, key):
            """ones-matmul partition reduce (head B at partition 32) then
            reciprocal into fp16 rrc."""
            accv = ptblk[:, 2, :]
            den = pp.tile([128, QW], F32, tag="pp", name=f"den_{key}")
            nc.tensor.matmul(den[0:1, :], onesK[:, 0:1], accv[:, 0:QW],
                             start=True, stop=True, tile_position=(0, 0))
            nc.tensor.matmul(den[32:33, :], onesK[:, 0:1],
                             accv[:, QW:2 * QW],
                             start=True, stop=True, tile_position=(0, 32))
            rrc = rrp.tile([1, 2 * QW], F32, tag="rrc", name=f"rrc_{key}")
            nc.vector.reciprocal_approx_fast(rrc[0:1, 0:QW], den[0:1, :])
            nc.vector.reciprocal_approx_fast(
                rrc[0:1, QW:2 * QW], den[32:33, :])
            st[key] = rrc

        def den_bc(p, qw_i, rrc, cop, key):
            bc = pp.tile([128, QW], F32, tag="pp", name=f"bc_{key}")
            nc.tensor.matmul(bc[0:64, :], ones1[0:1, 0:64], rrc[0:1, 0:QW],
                             start=True, stop=True, tile_position=(0, 0))
            nc.tensor.matmul(bc[64:128, :], ones1[0:1, 0:64],
                             rrc[0:1, QW:2 * QW],
                             start=True, stop=True, tile_position=(0, 64))
            bcs = rrp.tile([128, QW], F32, tag="bcs", name=f"bcs_{key}")
            nc.vector.tensor_copy(bcs[:], bc[:])
            nc.vector.tensor_mul(
                ctxP[p][:, qw_i * QW:(qw_i + 1) * QW], cop[:], bcs[:])

        def outproj_one(qw_i, c, evq=None, dmaq=None):
            tt, et = c // 2, c % 2
            t0 = qw_i * QW + tt * 128
            po = pp.tile([128, 512], F32, tag="pp",
                         name=f"po{qw_i}_{tt}_{et}")
            for p in range(2):
                nc.tensor.matmul(
                    po[:], ctxP[p][:, t0:t0 + 128],
                    wo_sb[p][:, et * 512:(et + 1) * 512],
                    start=(p == 0), stop=(p == 1))
            ev = oev.tile([128, 512], F16, tag="oev")
            if evq is nc.scalar:
                nc.scalar.copy(ev[:], po[:])
            else:
                nc.vector.tensor_copy(ev[:], po[:])
            (dmaq or nc.sync).dma_start(
                out=pout[t0:t0 + 128, et * 512:(et + 1) * 512],
                in_=ev[:])

        # ---------- DMA issue ----------
        # sync:   xq0a, wk, xk[512:1024], xk[1536:2048], out-DMA w0..w2
        # scalar: xq0b, xk[0:256], xk[256:512], xk[1024:1536], xq1..3
        # gpsimd: wq, wv, xv0..2, wo0, wo1, xv3
        xq_t = [None] * NQW
        xq_t[0] = dma_x_block(xqb, xqT, 0, QW, "xq0", halves=True)
        nc.gpsimd.dma_start(
            out=wq_sb, in_=wqT.rearrange("(ko ki) m -> ki ko m", ki=128))
        KW = [256, 256, 512, 512, 512]
        KC0 = [0, 256, 512, 1024, 1536]
        KQ = [nc.scalar, nc.scalar, nc.sync, nc.scalar, nc.sync]
        nc.sync.dma_start(
            out=wk_sb, in_=wkT.rearrange("(ko ki) m -> ki ko m", ki=128))
        xk_t = [dma_x_block(xkb if w == 512 else xkb2, xkT, c0, w,
                            f"xk{c0}", queue=q)
                for c0, w, q in zip(KC0, KW, KQ)]
        nc.gpsimd.dma_start(
            out=wv_sb, in_=wvT.rearrange("(ko ki) m -> ki ko m", ki=128))
        xv_t = [dma_x_block(xvb, xvT, b * CB, CB, f"xv{b}", queue=nc.gpsimd)
                for b in range(3)]
        nc.gpsimd.dma_start(out=wo_sb[0], in_=woTs[0:128, :])
        nc.gpsimd.dma_start(out=wo_sb[1], in_=woTs[128:256, :])
        xq_t[1] = dma_x_block(xqb, xqT, QW, QW, "xq1", queue=nc.scalar)
        xv_t.append(dma_x_block(xvb, xvT, 3 * CB, CB, "xv3",
                                queue=nc.gpsimd))
        xq_t[2] = dma_x_block(xqb, xqT, 2 * QW, QW, "xq2", queue=nc.scalar)
        xq_t[3] = dma_x_block(xqb, xqT, 3 * QW, QW, "xq3", queue=nc.scalar)

        # hold the PE clock gate open through the ramp's DMA shadow
        dmw = copp.tile([128, QW], F32, tag="cop", name="dm_ramp")
        def pe_warm(n):
            for _ in range(n):
                nc.tensor.matmul(dmw[0:1, 0:64], onesK[:, 0:1], wmt[:, :],
                                 start=True, stop=True)
        pe_warm(40)

        qt0 = qtp.tile([128, 2, QW], F16, tag="qt", name="qt0")
        qt1 = qtp.tile([128, 2, QW], F16, tag="qt", name="qt1")
        ptblk0 = ptp.tile([128, NKT, 2 * QW], F16, tag="pt", name="pt0")
        ptblk1 = ptp.tile([128, NKT, 2 * QW], F16, tag="pt", name="pt1")
        cop0 = copp.tile([128, QW], F32, tag="cop", name="cop0")
        cop1 = copp.tile([128, QW], F32, tag="cop", name="cop1")

        # ---------- ramp: block (0,p0), explicit schedule ----------
        C0 = lambda kt: chunk(0, kt, qt0, ptblk0, "r")
        proj_q_quarter(qt0, 0, xq_t[0], 0, "q0m0")
        proj_q_quarter(qt0, 0, xq_t[0], 1, "q0m0")
        pe_warm(10)
        proj_k_half(0, 256, 0, xk_t[0])
        C0(0)
        proj_q_quarter(qt0, 1, xq_t[0], 0, "q0m1")
        C0(1)
        proj_q_quarter(qt0, 1, xq_t[0], 1, "q0m1")
        proj_k_half(256, 256, 0, xk_t[1])
        C0(2)
        proj_k_half(0, 256, 1, xk_t[0])
        C0(3)
        proj_k_half(512, 512, 0, xk_t[2])
        C0(4)
        proj_k_half(256, 256, 1, xk_t[1])
        C0(5)
        C0(6)
        proj_k_half(1024, 512, 0, xk_t[3])
        C0(7)
        C0(8)
        proj_v_slice(0, 0, xv_t[0], 0)
        C0(9)
        proj_v_slice(0, 0, xv_t[0], 1)
        proj_k_half(512, 512, 1, xk_t[2])
        C0(10)
        proj_v_slice(0, 1, xv_t[0], 0)
        C0(11)
        proj_v_slice(0, 1, xv_t[0], 1)
        proj_k_half(1536, 512, 0, xk_t[4])
        C0(12)
        proj_v_slice(1, 0, xv_t[1], 0)
        C0(13)
        proj_v_slice(1, 0, xv_t[1], 1)
        proj_k_half(1024, 512, 1, xk_t[3])
        C0(14)
        proj_v_slice(1, 1, xv_t[1], 0)
        C0(15)

        # ---------- insert queue ----------
        queue = []  # (cost, fn)

        def pops(budget=1.35):
            spent = 0.0
            while queue and spent + queue[0][0] <= budget:
                cost, fn = queue.pop(0)
                fn()
                spent += cost

        def add(cost, fn):
            queue.append((cost, fn))

        # remaining ramp leftovers -> head of queue for the fill era
        add(0.9, lambda: proj_v_slice(1, 1, xv_t[1], 1))
        add(0.9, lambda: proj_k_half(1536, 512, 1, xk_t[4]))

        # ---------- fill era: block (0,p1) ----------
        # items: remaining V, ctx(0,p0), den(0,p0), qt1-m0.  The den tree
        # writes pt slots 0,1,4,5 (lo) and 2,3,6..9 (hi): it must trail the
        # ctx duos that read those slots (d0,d2 before lo; d1,d3,d4 before
        # hi) -- FIFO pop order preserves this.
        add(0.5, lambda: emit_ctx_duo(0, 0, ptblk0, cop0))
        add(0.5, lambda: emit_ctx_duo(0, 1, ptblk0, cop0))
        add(0.9, lambda: proj_v_slice(2, 0, xv_t[2], 0))
        add(0.9, lambda: proj_v_slice(2, 0, xv_t[2], 1))
        add(0.5, lambda: emit_ctx_duo(0, 2, ptblk0, cop0))
        add(0.1, lambda: tree_lo(ptblk0, "lo_00"))
        add(0.9, lambda: proj_v_slice(2, 1, xv_t[2], 0))
        add(0.9, lambda: proj_v_slice(2, 1, xv_t[2], 1))
        add(0.5, lambda: emit_ctx_duo(0, 3, ptblk0, cop0))
        add(0.5, lambda: emit_ctx_duo(0, 4, ptblk0, cop0))
        add(0.1, lambda: tree_hi(ptblk0, "lo_00"))
        add(0.9, lambda: proj_v_slice(3, 0, xv_t[3], 0))
        add(0.9, lambda: proj_v_slice(3, 0, xv_t[3], 1))
        add(0.5, lambda: emit_ctx_duo(0, 5, ptblk0, cop0))
        add(0.15, lambda: den_mm(ptblk0, "rrc_00"))
        add(0.9, lambda: proj_v_slice(3, 1, xv_t[3], 0))
        add(0.9, lambda: proj_v_slice(3, 1, xv_t[3], 1))
        add(0.5, lambda: emit_ctx_duo(0, 6, ptblk0, cop0))
        add(0.5, lambda: emit_ctx_duo(0, 7, ptblk0, cop0))
        add(0.15, lambda: den_bc(0, 0, st["rrc_00"], cop0, "00"))
        add(0.85, lambda: proj_q_quarter(qt1, 0, xq_t[1], 0, "q1m0"))
        add(0.85, lambda: proj_q_quarter(qt1, 0, xq_t[1], 1, "q1m0"))

        for kt_i in range(NKT):
            chunk(1, kt_i, qt0, ptblk1, "f")
            pops()

        # ---------- steady blocks ----------
        blocks = [(qw_i, p) for qw_i in range(NQW) for p in range(2)][2:]
        prev = (1, 0, ptblk1, cop1, list(range(8)), "01")  # p,qw,pt,cop,duos
        qt_cur = qt1
        qt_nxt = None
        for bi, (qw_i, p) in enumerate(blocks):
            blk_i = bi + 2
            last = bi == len(blocks) - 1
            key = f"{qw_i}{p}"
            ptblk = ptp.tile([128, NKT, 2 * QW], F16, tag="pt",
                             name=f"pt{blk_i}")
            cop = copp.tile([128, QW], F32, tag="cop", name=f"cop{blk_i}")
            qt_b = qt_cur
            pp_, pqw, ppt, pcop, pduos, pkey = prev
            if p == 1 and qw_i + 1 < NQW:
                qt_nxt = qtp.tile([128, 2, QW], F16, tag="qt",
                                  name=f"qt{qw_i + 1}")
            # force-drain before this block's chunks: leftover items may
            # include this block's qt producers (PE in-order: a score that
            # waits on a later-emitted PE producer would deadlock)
            pops(budget=99.0)
            for jj in range(4):
                if jj == 0:
                    for d in pduos:
                        add(0.5, lambda a=pp_, d=d, t=ppt, c=pcop:
                            emit_ctx_duo(a, d, t, c))
                    if bi == 0:
                        # qt1-m1 for the sibling block (1,p1)
                        add(0.85, lambda: proj_q_quarter(
                            qt1, 1, xq_t[1], 0, "q1m1"))
                        add(0.85, lambda: proj_q_quarter(
                            qt1, 1, xq_t[1], 1, "q1m1"))
                    if p == 1 and qw_i + 1 < NQW:
                        nq = qw_i + 1
                        add(0.85, lambda nq=nq, qn=qt_nxt: proj_q_quarter(
                            qn, 0, xq_t[nq], 0, f"q{nq}m0"))
                        add(0.85, lambda nq=nq, qn=qt_nxt: proj_q_quarter(
                            qn, 0, xq_t[nq], 1, f"q{nq}m0"))
                if jj == 1:
                    add(0.1, lambda t=ppt, k=pkey: tree_lo(t, "lo_" + k))
                    add(0.1, lambda t=ppt, k=pkey: tree_hi(t, "lo_" + k))
                    add(0.5, lambda p=p, t=ptblk, c=cop:
                        emit_ctx_duo(p, 0, t, c))
                    add(0.5, lambda p=p, t=ptblk, c=cop:
                        emit_ctx_duo(p, 1, t, c))
                    if p == 1 and qw_i + 1 < NQW:
                        nq = qw_i + 1
                        add(0.85, lambda nq=nq, qn=qt_nxt: proj_q_quarter(
                            qn, 1, xq_t[nq], 0, f"q{nq}m1"))
                        add(0.85, lambda nq=nq, qn=qt_nxt: proj_q_quarter(
                            qn, 1, xq_t[nq], 1, f"q{nq}m1"))
                if jj == 2:
                    add(0.15, lambda t=ppt, k=pkey: den_mm(t, "rrc_" + k))
                    add(0.5, lambda p=p, t=ptblk, c=cop:
                        emit_ctx_duo(p, 2, t, c))
                    add(0.5, lambda p=p, t=ptblk, c=cop:
                        emit_ctx_duo(p, 3, t, c))
                    if p == 1 and qw_i >= 1:
                        for c in range(4):
                            add(0.5, lambda w=qw_i - 1, c=c:
                                outproj_one(w, c))
                    if last:
                        add(0.1, lambda t=ptblk: tree_lo(t, "lo_l"))
                if jj == 3:
                    add(0.15, lambda a=pp_, w=pqw, c=pcop, k=pkey:
                        den_bc(a, w, st["rrc_" + k], c, k))
                    add(0.5, lambda p=p, t=ptblk, c=cop:
                        emit_ctx_duo(p, 4, t, c))
                    add(0.5, lambda p=p, t=ptblk, c=cop:
                        emit_ctx_duo(p, 5, t, c))
                    if p == 1 and qw_i >= 1:
                        for c in range(4, 8):
                            add(0.5, lambda w=qw_i - 1, c=c:
                                outproj_one(w, c))
                    if last:
                        def last_partials(t=ptblk):
                            pl = lambda a, b: t[:, a:b, :]
                            stl = st["lo_l"]
                            nc.vector.tensor_add(pl(2, 4), pl(8, 10),
                                                 pl(10, 12))       # C: 8..11
                            nc.vector.tensor_add(pl(0, 2), pl(4, 6),
                                                 pl(2, 4))         # E+C
                            nc.vector.tensor_add(
                                stl[:, 0, :], t[:, 0, :],
                                t[:, 1, :])                        # kt0..11
                            nc.vector.tensor_add(
                                stl[:, 1, :], t[:, 12, :],
                                t[:, 13, :])                       # 12+13
                            nc.vector.tensor_add(
                                t[:, 0, :], stl[:, 0, :],
                                stl[:, 1, :])                      # S: 0..13
                            nc.vector.tensor_add(
                                t[:, 1, :], t[:, 0, :],
                                t[:, 14, :])                       # S': 0..14
                        add(0.1, last_partials)
                for half in range(2):
                    for kt_i in (4 * jj + 2 * half, 4 * jj + 2 * half + 1):
                        chunk(p, kt_i, qt_b, ptblk, f"s{blk_i}")
                    pops()
            prev = (p, qw_i, ptblk, cop, [6, 7], key)
            if p == 1 and qw_i + 1 < NQW:
                qt_cur = qt_nxt

        # ---------- tail ----------
        p_l, qw_l, ptblk_l, cop_l = prev[0], prev[1], prev[2], prev[3]
        dm = scp.tile([128, 2 * QW], F32, tag="sc", name="dm_warm")
        warm_mm = lambda: nc.tensor.matmul(
            dm[0:1, 0:64], onesK[:, 0:1], wmt[:, :], start=True, stop=True)
        # drain whatever is left (own duos 4,5 / po(w2) tail / den partials)
        emit_ctx_duo(p_l, 6, ptblk_l, cop_l)
        pops(budget=99.0)
        warm_mm()
        nc.vector.tensor_add(ptblk_l[:, 2, :], ptblk_l[:, 1, :],
                             ptblk_l[:, 15, :])          # acc = S' + kt15
        emit_ctx(p_l, 14, ptblk_l, cop_l)
        emit_ctx(p_l, 15, ptblk_l, cop_l)
        den_mm(ptblk_l, "rrc_l")
        warm_mm()
        rrc_l = st["rrc_l"]
        bc = pp.tile([128, QW], F32, tag="pp", name="bc_l")
        nc.tensor.matmul(bc[0:64, :], ones1[0:1, 0:64], rrc_l[0:1, 0:QW],
                         start=True, stop=True, tile_position=(0, 0))
        nc.tensor.matmul(bc[64:128, :], ones1[0:1, 0:64],
                         rrc_l[0:1, QW:2 * QW],
                         start=True, stop=True, tile_position=(0, 64))
        bcs = rrp.tile([128, QW], F32, tag="bcs", name="bcs_l")
        nc.vector.tensor_copy(bcs[:], bc[:])
        for tt in range(4):
            c0, c1 = tt * 128, (tt + 1) * 128
            nc.vector.tensor_mul(
                ctxP[p_l][:, qw_l * QW + c0:qw_l * QW + c1],
                cop_l[:, c0:c1], bcs[:, c0:c1])
            outproj_one(NQW - 1, 2 * tt, evq=nc.vector, dmaq=nc.sync)
            outproj_one(NQW - 1, 2 * tt + 1, evq=nc.scalar, dmaq=nc.scalar)

    nc.compile()
    return nc


def kernel(query, key, value, Wq, Wk, Wv, Wo):
    global _PROG, _LAST_IN_MAPS
    from concourse.bass_utils import run_bass_kernel_spmd

    if _PROG is None:
        _PROG = _build()
    nc = _PROG

    q2 = np.asarray(query, dtype=np.float32).reshape(B, S, D)
    k2 = np.asarray(key, dtype=np.float32).reshape(B, S, D)
    v2 = np.asarray(value, dtype=np.float32).reshape(B, S, D)
    Wq = np.asarray(Wq, dtype=np.float32)
    Wk = np.asarray(Wk, dtype=np.float32)
    Wv = np.asarray(Wv, dtype=np.float32)
    Wo = np.asarray(Wo, dtype=np.float32)

    xT = {}
    for b in range(B):
        xT[("q", b)] = np.ascontiguousarray(q2[b].T).astype(np.float16)
        xT[("k", b)] = np.ascontiguousarray(k2[b].T).astype(np.float16)
        xT[("v", b)] = np.ascontiguousarray(v2[b].T).astype(np.float16)

    in_maps = []
    for c in range(NCORES):
        b = c // 4
        l = c % 4
        rs = slice(CHD * l, CHD * (l + 1))
        in_maps.append({
            "xqT": xT[("q", b)],
            "xkT": xT[("k", b)],
            "xvT": xT[("v", b)],
            "wqT": (Wq[rs, :].T * SCALE).astype(np.float16),
            "wkT": Wk[rs, :].T.astype(np.float16),
            "wvT": Wv[rs, :].T.astype(np.float16),
            "woTs": np.ascontiguousarray(Wo[:, rs].T).astype(np.float16),
        })

    _LAST_IN_MAPS = in_maps
    res = run_bass_kernel_spmd(nc, in_maps, core_ids=list(range(NCORES)))
    parts = [res.results[c]["pout"].astype(np.float32) for c in range(NCORES)]
    out = np.empty((B, S, D), dtype=np.float32)
    for b in range(B):
        out[b] = parts[4 * b] + parts[4 * b + 1] + parts[4 * b + 2] + parts[4 * b + 3]
    return out


# revision 12
# speedup vs baseline: 1.1369x; 1.1369x over previous
"""Trainium2 Bass kernel for nn_MultiHeadAttention (B=2, S=2048, D=1024, H=16).

Sharding: 8 cores = 2 batch groups x 4 cores. Core c handles batch c//4 and
heads 4*(c%4) .. 4*(c%4)+4 (two head-pairs p=0,1). Each core computes Q/K/V
projections for its batch+heads, transposed-layout attention, and a partial
output projection over its 256 head-dims. Host sums the 4 partials per batch.

v3 design (vs v2 baseline @211.7us):
- ACT exp is the roofline (~143us busy): the schedule aims to start the exp
  stream ASAP and never starve it.
- 3 parallel DMA queues (sync/scalar HWDGE + gpsimd SWDGE); the ramp's
  critical Q/K loads split across queues (xq0 in two half-DMAs).
- all non-score PE work (projections, ctx, den matmuls, out-proj) is sliced
  into <~1us items in a FIFO insert queue, popped between score-chunk pairs
  with a per-point cost budget: the 2-deep sc PSUM ring only buffers 2.2us
  of ACT runway, so no insert slab may exceed it.
- den/bc broadcast matmuls in fp16 (were fp32 LOW_HIGH, 3x slower); the
  den->recip->bc chain is split across points so PE never waits on DVE.
- den tree adds C,D on the otherwise-idle gpsimd (DVE was co-bottleneck).
- tail: den pre-summed to one post-exp add, evictions split DVE/ACT,
  out-DMA split over both HWDGE queues, PE kept warm.
"""

import numpy as np

B, S, D, H = 2, 2048, 1024, 16
HD = D // H          # 64
NCORES = 8
HPC = 4              # heads per core
CHD = HPC * HD       # 256 head-dims per core
TOK = S              # tokens per core (one batch)
QW = 512             # query window
NQW = TOK // QW      # 4 windows
NKT = TOK // 128     # 16 key tiles
SCALE = 1.0 / np.sqrt(np.float32(D))  # 1/32, folded into Wq on host

_PROG = None
_LAST_IN_MAPS = None


def _build():
    from contextlib import ExitStack

    import concourse.bass as bass
    import concourse.tile as tile
    from concourse import bacc, mybir

    F16 = mybir.dt.float16
    F32 = mybir.dt.float32
    EXP = mybir.ActivationFunctionType.Exp

    nc = bacc.Bacc("TRN2", target_bir_lowering=False, debug=False,
                   num_devices=NCORES)

    xqT = nc.dram_tensor("xqT", [D, TOK], F16, kind="ExternalInput").ap()
    xkT = nc.dram_tensor("xkT", [D, TOK], F16, kind="ExternalInput").ap()
    xvT = nc.dram_tensor("xvT", [D, TOK], F16, kind="ExternalInput").ap()
    wqT = nc.dram_tensor("wqT", [D, CHD], F16, kind="ExternalInput").ap()
    wkT = nc.dram_tensor("wkT", [D, CHD], F16, kind="ExternalInput").ap()
    wvT = nc.dram_tensor("wvT", [D, CHD], F16, kind="ExternalInput").ap()
    woTs = nc.dram_tensor("woTs", [CHD, D], F16, kind="ExternalInput").ap()
    pout = nc.dram_tensor("pout", [TOK, D], F16, kind="ExternalOutput").ap()

    CB = 512            # V column-block width (tokens)

    with tile.TileContext(nc) as tc, ExitStack() as ctx:
        const = ctx.enter_context(tc.tile_pool(name="const", bufs=1))
        wq_sb = const.tile([128, 8, CHD], F16, tag="wq")
        wk_sb = const.tile([128, 8, CHD], F16, tag="wk")
        wv_sb = const.tile([128, 8, CHD], F16, tag="wv")
        wo_sb = [const.tile([128, D], F16, tag=f"wo{p}", name=f"wo{p}")
                 for p in range(2)]
        onesK = const.tile([128, 1], F16, tag="onesK")
        ones1 = const.tile([1, 128], F32, tag="ones1")

        nc.vector.memset(onesK, 1.0)
        nc.vector.memset(ones1, 1.0)

        warm = const.tile([1, 8], F32, tag="warm")
        nc.vector.memset(warm, 0.0)
        nc.scalar.activation(out=warm, in_=warm, func=EXP)
        wmt = const.tile([128, 64], F16, tag="wmt")
        nc.vector.memset(wmt, 1.0)

        big = ctx.enter_context(tc.tile_pool(name="big", bufs=1))
        KT = big.tile([128, 2, TOK], F16, tag="kt")          # [hd, pair, keys]
        vnat = big.tile([128, NKT, CHD], F16, tag="vnat")    # [keys, kt, hd]
        ctxP = [big.tile([128, TOK], F16, tag=f"ctxP{p}", name=f"ctxP{p}")
                for p in range(2)]

        xkb = ctx.enter_context(tc.tile_pool(name="xkb", bufs=3))
        xkb2 = ctx.enter_context(tc.tile_pool(name="xkb2", bufs=2))
        xvb = ctx.enter_context(tc.tile_pool(name="xvb", bufs=3))
        xqb = ctx.enter_context(tc.tile_pool(name="xqb", bufs=2))
        qtp = ctx.enter_context(tc.tile_pool(name="qtp", bufs=2))
        ptp = ctx.enter_context(tc.tile_pool(name="ptp", bufs=2))
        dtp = ctx.enter_context(tc.tile_pool(name="dtp", bufs=2))
        rrp = ctx.enter_context(tc.tile_pool(name="rrp", bufs=2))
        oev = ctx.enter_context(tc.tile_pool(name="oev", bufs=4))

        # PSUM (8 banks): sc ring 2x[128,1024] (4) + cop ring 2x[128,512] (2)
        # + shared proj/out/den/bcast pool 2x[128,512] (2)
        scp = ctx.enter_context(tc.tile_pool(name="scp", bufs=2, space="PSUM"))
        copp = ctx.enter_context(tc.tile_pool(name="copp", bufs=2, space="PSUM"))
        pp = ctx.enter_context(tc.tile_pool(name="pp", bufs=2, space="PSUM"))

        st = {}  # cross-item state (psum tiles, tree accumulators, rrc)

        # ---------- emission helpers ----------
        def dma_x_block(pool, src, c0, w, name, queue=None, halves=False):
            t = pool.tile([128, 8, w], F16, tag="xb", name=name)
            r = src[:, c0:c0 + w].rearrange("(ko ki) t -> ki ko t", ki=128)
            if halves:
                nc.sync.dma_start(out=t[:, 0:4, :], in_=r[:, 0:4, :])
                nc.scalar.dma_start(out=t[:, 4:8, :], in_=r[:, 4:8, :])
            else:
                (queue or nc.sync).dma_start(out=t, in_=r)
            return t

        def proj_q_quarter(qt, m, xq_t, phase, name):
            if phase == 0:
                st[name] = pp.tile([128, QW], F32, tag="pp", name=name)
            psq = st[name]
            for ko in range(4 * phase, 4 * phase + 4):
                nc.tensor.matmul(
                    psq[:], wq_sb[:, ko, m * 128:(m + 1) * 128],
                    xq_t[:, ko, :], start=(ko == 0), stop=(ko == 7))
            if phase == 1:
                nc.vector.tensor_copy(qt[:, m, :], psq[:])
                del st[name]

        def proj_k_half(c0, w, m, xk_t):
            psk = pp.tile([128, w], F32, tag="pp", name=f"psk{c0}_{m}")
            for ko in range(8):
                nc.tensor.matmul(
                    psk[:], wk_sb[:, ko, m * 128:(m + 1) * 128],
                    xk_t[:, ko, :], start=(ko == 0), stop=(ko == 7))
            nc.vector.tensor_copy(KT[:, m, c0:c0 + w], psk[:])

        def proj_v_slice(blk, half, xv_t, tt):
            name = f"pv{blk}_{half}"
            if tt == 0:
                st[name] = pp.tile([128, 2 * CHD], F32, tag="pp", name=name)
            pv = st[name]
            t0 = (2 * half + tt) * 128
            for ko in range(8):
                nc.tensor.matmul(
                    pv[:, tt * CHD:(tt + 1) * CHD],
                    xv_t[:, ko, t0:t0 + 128],
                    wv_sb[:, ko, :], start=(ko == 0), stop=(ko == 7))
            if tt == 1:
                nc.vector.tensor_copy(
                    vnat[:, 4 * blk + 2 * half:4 * blk + 2 * half + 2, :],
                    bass.AP(tensor=pv.tensor, offset=pv.offset,
                            ap=[list(pv.ap[0]), [CHD, 2], [1, CHD]]))
                del st[name]

        def chunk(p, kt_i, qt, ptblk, tag=""):
            k0 = kt_i * 128
            sc = scp.tile([128, 2 * QW], F32, tag="sc",
                          name=f"sc{tag}_{kt_i}")
            nc.tensor.matmul(
                sc[:, 0:QW], KT[0:64, p, k0:k0 + 128], qt[0:64, p, :],
                start=True, stop=True, tile_position=(0, 0))
            nc.tensor.matmul(
                sc[:, QW:2 * QW], KT[64:128, p, k0:k0 + 128], qt[64:128, p, :],
                start=True, stop=True, tile_position=(64, 0))
            nc.scalar.activation(out=ptblk[:, kt_i, :], in_=sc[:], func=EXP)

        def emit_ctx(p, kt_i, ptblk, cop):
            h0 = p * 128
            nc.tensor.matmul(
                cop[0:64, :], vnat[:, kt_i, h0:h0 + 64],
                ptblk[:, kt_i, 0:QW],
                start=(kt_i == 0), stop=(kt_i == NKT - 1),
                tile_position=(0, 0))
            nc.tensor.matmul(
                cop[64:128, :], vnat[:, kt_i, h0 + 64:h0 + 128],
                ptblk[:, kt_i, QW:2 * QW],
                start=(kt_i == 0), stop=(kt_i == NKT - 1),
                tile_position=(0, 64))

        def emit_ctx_duo(p, d, ptblk, cop):
            emit_ctx(p, 2 * d, ptblk, cop)
            emit_ctx(p, 2 * d + 1, ptblk, cop)

        def tree_lo(ptblk, key):
            stt = dtp.tile([128, 2, 2 * QW], F16, tag="dt", name=f"dt_{key}")
            pv = lambda a, b: ptblk[:, a:b, :]
            nc.vector.tensor_add(stt[:, 0:2, :], pv(0, 2), pv(2, 4))    # A
            nc.vector.tensor_add(pv(0, 2), pv(4, 6), pv(6, 8))          # B
            nc.vector.tensor_add(pv(4, 6), stt[:, 0:2, :], pv(0, 2))    # E
            st[key] = stt

        def tree_hi(ptblk, key):
            pv = lambda a, b: ptblk[:, a:b, :]
            nc.gpsimd.tensor_add(pv(2, 4), pv(8, 10), pv(10, 12))       # C
            nc.gpsimd.tensor_add(pv(6, 8), pv(12, 14), pv(14, 16))      # D
            nc.vector.tensor_add(pv(8, 10), pv(2, 4), pv(6, 8))         # F
            nc.vector.tensor_add(pv(0, 2), pv(4, 6), pv(8, 10))         # G
            nc.vector.tensor_add(ptblk[:, 2, :], ptblk[:, 0, :],
                                 ptblk[:, 1, :])                        # acc

        def den_mm(ptblk, key):
            """ones-matmul partition reduce per head, reciprocal into rrc."""
            accv = ptblk[:, 2, :]
            rrc = rrp.tile([1, 2 * QW], F32, tag="rrc", name=f"rrc_{key}")
            for h in range(2):
                den = pp.tile([128, QW], F32, tag="pp", name=f"den_{key}_{h}")
                nc.tensor.matmul(den[0:1, :], onesK[:, 0:1],
                                 accv[:, h * QW:(h + 1) * QW],
                                 start=True, stop=True)
                nc.vector.reciprocal_approx_fast(
                    rrc[0:1, h * QW:(h + 1) * QW], den[0:1, :])
            st[key] = rrc

        def den_bc(p, qw_i, rrc, cop, key):
            bc = pp.tile([128, QW], F32, tag="pp", name=f"bc_{key}")
            nc.tensor.matmul(bc[0:64, :], ones1[0:1, 0:64], rrc[0:1, 0:QW],
                             start=True, stop=True, tile_position=(0, 0))
            nc.tensor.matmul(bc[64:128, :], ones1[0:1, 0:64],
                             rrc[0:1, QW:2 * QW],
                             start=True, stop=True, tile_position=(0, 64))
            bcs = rrp.tile([128, QW], F32, tag="bcs", name=f"bcs_{key}")
            nc.vector.tensor_copy(bcs[:], bc[:])
            nc.vector.tensor_mul(
                ctxP[p][:, qw_i * QW:(qw_i + 1) * QW], cop[:], bcs[:])

        def outproj_one(qw_i, c, evq=None, dmaq=None):
            tt, et = c // 2, c % 2
            t0 = qw_i * QW + tt * 128
            po = pp.tile([128, 512], F32, tag="pp",
                         name=f"po{qw_i}_{tt}_{et}")
            for p in range(2):
                nc.tensor.matmul(
                    po[:], ctxP[p][:, t0:t0 + 128],
                    wo_sb[p][:, et * 512:(et + 1) * 512],
                    start=(p == 0), stop=(p == 1))
            ev = oev.tile([128, 512], F16, tag="oev")
            if evq is nc.scalar:
                nc.scalar.copy(ev[:], po[:])
            else:
                nc.vector.tensor_copy(ev[:], po[:])
            (dmaq or nc.sync).dma_start(
                out=pout[t0:t0 + 128, et * 512:(et + 1) * 512],
                in_=ev[:])

        # ---------- DMA issue ----------
        # sync:   xq0a, wk, xk[512:1024], xk[1536:2048], out-DMA w0..w2
        # scalar: xq0b, xk[0:256], xk[256:512], xk[1024:1536], xq1..3
        # gpsimd: wq, wv, xv0..2, wo0, wo1, xv3
        xq_t = [None] * NQW
        xq_t[0] = dma_x_block(xqb, xqT, 0, QW, "xq0", halves=True)
        nc.gpsimd.dma_start(
            out=wq_sb, in_=wqT.rearrange("(ko ki) m -> ki ko m", ki=128))
        KW = [256, 256, 512, 512, 512]
        KC0 = [0, 256, 512, 1024, 1536]
        KQ = [nc.scalar, nc.scalar, nc.sync, nc.scalar, nc.sync]
        nc.sync.dma_start(
            out=wk_sb, in_=wkT.rearrange("(ko ki) m -> ki ko m", ki=128))
        xk_t = [dma_x_block(xkb if w == 512 else xkb2, xkT, c0, w,
                            f"xk{c0}", queue=q)
                for c0, w, q in zip(KC0, KW, KQ)]

        # hold the PE clock gate open through the ramp's DMA shadow
        dmw = copp.tile([128, QW], F32, tag="cop", name="dm_ramp")
        def pe_warm(n):
            for _ in range(n):
                nc.tensor.matmul(dmw[0:1, 0:64], onesK[:, 0:1], wmt[:, :],
                                 start=True, stop=True)
        pe_warm(70)

        qt0 = qtp.tile([128, 2, QW], F16, tag="qt", name="qt0")
        qt1 = qtp.tile([128, 2, QW], F16, tag="qt", name="qt1")
        ptblk0 = ptp.tile([128, NKT, 2 * QW], F16, tag="pt", name="pt0")
        ptblk1 = ptp.tile([128, NKT, 2 * QW], F16, tag="pt", name="pt1")
        cop0 = copp.tile([128, QW], F32, tag="cop", name="cop0")
        cop1 = copp.tile([128, QW], F32, tag="cop", name="cop1")

        # ---------- ramp: block (0,p0), explicit schedule ----------
        C0 = lambda kt: chunk(0, kt, qt0, ptblk0, "r")
        proj_q_quarter(qt0, 0, xq_t[0], 0, "q0m0")
        proj_q_quarter(qt0, 0, xq_t[0], 1, "q0m0")
        # gpsimd-queue guard: V/wo loads (4.5MB) must not steal HBM
        # bandwidth from the ramp-critical Q/K loads (SDMA round-robins
        # across queues).  The tiny copy below waits for the qt0-m0
        # eviction, stalling the gpsimd DGE until the critical window ends.
        gscr = const.tile([128, 8], F16, tag="gscr")
        nc.gpsimd.tensor_copy(gscr[:], qt0[:, 0, 0:8])
        nc.gpsimd.dma_start(
            out=wv_sb, in_=wvT.rearrange("(ko ki) m -> ki ko m", ki=128))
        xv_t = [dma_x_block(xvb, xvT, b * CB, CB, f"xv{b}", queue=nc.gpsimd)
                for b in range(3)]
        nc.gpsimd.dma_start(out=wo_sb[0], in_=woTs[0:128, :])
        nc.gpsimd.dma_start(out=wo_sb[1], in_=woTs[128:256, :])
        xq_t[1] = dma_x_block(xqb, xqT, QW, QW, "xq1", queue=nc.scalar)
        xv_t.append(dma_x_block(xvb, xvT, 3 * CB, CB, "xv3",
                                queue=nc.gpsimd))
        xq_t[2] = dma_x_block(xqb, xqT, 2 * QW, QW, "xq2", queue=nc.scalar)
        xq_t[3] = dma_x_block(xqb, xqT, 3 * QW, QW, "xq3", queue=nc.scalar)
        pe_warm(25)
        proj_k_half(0, 256, 0, xk_t[0])
        C0(0)
        proj_q_quarter(qt0, 1, xq_t[0], 0, "q0m1")
        C0(1)
        proj_q_quarter(qt0, 1, xq_t[0], 1, "q0m1")
        proj_k_half(256, 256, 0, xk_t[1])
        C0(2)
        proj_k_half(0, 256, 1, xk_t[0])
        C0(3)
        proj_k_half(512, 512, 0, xk_t[2])
        C0(4)
        proj_k_half(256, 256, 1, xk_t[1])
        C0(5)
        C0(6)
        proj_k_half(1024, 512, 0, xk_t[3])
        C0(7)
        C0(8)
        proj_v_slice(0, 0, xv_t[0], 0)
        C0(9)
        proj_v_slice(0, 0, xv_t[0], 1)
        proj_k_half(512, 512, 1, xk_t[2])
        C0(10)
        proj_v_slice(0, 1, xv_t[0], 0)
        C0(11)
        proj_v_slice(0, 1, xv_t[0], 1)
        proj_k_half(1536, 512, 0, xk_t[4])
        C0(12)
        proj_v_slice(1, 0, xv_t[1], 0)
        C0(13)
        proj_v_slice(1, 0, xv_t[1], 1)
        proj_k_half(1024, 512, 1, xk_t[3])
        C0(14)
        proj_v_slice(1, 1, xv_t[1], 0)
        C0(15)

        # ---------- insert queue ----------
        queue = []  # (cost, fn)

        def pops(budget=1.35):
            spent = 0.0
            while queue and spent + queue[0][0] <= budget:
                cost, fn = queue.pop(0)
                fn()
                spent += cost

        def add(cost, fn):
            queue.append((cost, fn))

        # remaining ramp leftovers -> head of queue for the fill era
        add(0.9, lambda: proj_v_slice(1, 1, xv_t[1], 1))
        add(0.9, lambda: proj_k_half(1536, 512, 1, xk_t[4]))

        # ---------- fill era: block (0,p1) ----------
        # items: remaining V, ctx(0,p0), den(0,p0), qt1-m0.  The den tree
        # writes pt slots 0,1,4,5 (lo) and 2,3,6..9 (hi): it must trail the
        # ctx duos that read those slots (d0,d2 before lo; d1,d3,d4 before
        # hi) -- FIFO pop order preserves this.
        add(0.5, lambda: emit_ctx_duo(0, 0, ptblk0, cop0))
        add(0.5, lambda: emit_ctx_duo(0, 1, ptblk0, cop0))
        add(0.9, lambda: proj_v_slice(2, 0, xv_t[2], 0))
        add(0.9, lambda: proj_v_slice(2, 0, xv_t[2], 1))
        add(0.5, lambda: emit_ctx_duo(0, 2, ptblk0, cop0))
        add(0.1, lambda: tree_lo(ptblk0, "lo_00"))
        add(0.9, lambda: proj_v_slice(2, 1, xv_t[2], 0))
        add(0.9, lambda: proj_v_slice(2, 1, xv_t[2], 1))
        add(0.5, lambda: emit_ctx_duo(0, 3, ptblk0, cop0))
        add(0.5, lambda: emit_ctx_duo(0, 4, ptblk0, cop0))
        add(0.1, lambda: tree_hi(ptblk0, "lo_00"))
        add(0.9, lambda: proj_v_slice(3, 0, xv_t[3], 0))
        add(0.9, lambda: proj_v_slice(3, 0, xv_t[3], 1))
        add(0.5, lambda: emit_ctx_duo(0, 5, ptblk0, cop0))
        add(0.15, lambda: den_mm(ptblk0, "rrc_00"))
        add(0.9, lambda: proj_v_slice(3, 1, xv_t[3], 0))
        add(0.9, lambda: proj_v_slice(3, 1, xv_t[3], 1))
        add(0.5, lambda: emit_ctx_duo(0, 6, ptblk0, cop0))
        add(0.5, lambda: emit_ctx_duo(0, 7, ptblk0, cop0))
        add(0.15, lambda: den_bc(0, 0, st["rrc_00"], cop0, "00"))
        add(0.85, lambda: proj_q_quarter(qt1, 0, xq_t[1], 0, "q1m0"))
        add(0.85, lambda: proj_q_quarter(qt1, 0, xq_t[1], 1, "q1m0"))

        for kt_i in range(NKT):
            chunk(1, kt_i, qt0, ptblk1, "f")
            pops()

        # ---------- steady blocks ----------
        blocks = [(qw_i, p) for qw_i in range(NQW) for p in range(2)][2:]
        prev = (1, 0, ptblk1, cop1, list(range(8)), "01")  # p,qw,pt,cop,duos
        qt_cur = qt1
        qt_nxt = None
        for bi, (qw_i, p) in enumerate(blocks):
            blk_i = bi + 2
            last = bi == len(blocks) - 1
            key = f"{qw_i}{p}"
            ptblk = ptp.tile([128, NKT, 2 * QW], F16, tag="pt",
                             name=f"pt{blk_i}")
            cop = copp.tile([128, QW], F32, tag="cop", name=f"cop{blk_i}")
            qt_b = qt_cur
            pp_, pqw, ppt, pcop, pduos, pkey = prev
            if p == 1 and qw_i + 1 < NQW:
                qt_nxt = qtp.tile([128, 2, QW], F16, tag="qt",
                                  name=f"qt{qw_i + 1}")
            # force-drain before this block's chunks: leftover items may
            # include this block's qt producers (PE in-order: a score that
            # waits on a later-emitted PE producer would deadlock)
            pops(budget=99.0)
            for jj in range(4):
                if jj == 0:
                    for d in pduos:
                        add(0.5, lambda a=pp_, d=d, t=ppt, c=pcop:
                            emit_ctx_duo(a, d, t, c))
                    if bi == 0:
                        # qt1-m1 for the sibling block (1,p1)
                        add(0.85, lambda: proj_q_quarter(
                            qt1, 1, xq_t[1], 0, "q1m1"))
                        add(0.85, lambda: proj_q_quarter(
                            qt1, 1, xq_t[1], 1, "q1m1"))
                    if p == 1 and qw_i + 1 < NQW:
                        nq = qw_i + 1
                        add(0.85, lambda nq=nq, qn=qt_nxt: proj_q_quarter(
                            qn, 0, xq_t[nq], 0, f"q{nq}m0"))
                        add(0.85, lambda nq=nq, qn=qt_nxt: proj_q_quarter(
                            qn, 0, xq_t[nq], 1, f"q{nq}m0"))
                if jj == 1:
                    add(0.1, lambda t=ppt, k=pkey: tree_lo(t, "lo_" + k))
                    add(0.1, lambda t=ppt, k=pkey: tree_hi(t, "lo_" + k))
                    add(0.5, lambda p=p, t=ptblk, c=cop:
                        emit_ctx_duo(p, 0, t, c))
                    add(0.5, lambda p=p, t=ptblk, c=cop:
                        emit_ctx_duo(p, 1, t, c))
                    if p == 1 and qw_i + 1 < NQW:
                        nq = qw_i + 1
                        add(0.85, lambda nq=nq, qn=qt_nxt: proj_q_quarter(
                            qn, 1, xq_t[nq], 0, f"q{nq}m1"))
                        add(0.85, lambda nq=nq, qn=qt_nxt: proj_q_quarter(
                            qn, 1, xq_t[nq], 1, f"q{nq}m1"))
                if jj == 2:
                    add(0.15, lambda t=ppt, k=pkey: den_mm(t, "rrc_" + k))
                    add(0.5, lambda p=p, t=ptblk, c=cop:
                        emit_ctx_duo(p, 2, t, c))
                    add(0.5, lambda p=p, t=ptblk, c=cop:
                        emit_ctx_duo(p, 3, t, c))
                    if p == 1 and qw_i >= 1:
                        for c in range(4):
                            add(0.5, lambda w=qw_i - 1, c=c:
                                outproj_one(w, c))
                    if last:
                        add(0.1, lambda t=ptblk: tree_lo(t, "lo_l"))
                if jj == 3:
                    add(0.15, lambda a=pp_, w=pqw, c=pcop, k=pkey:
                        den_bc(a, w, st["rrc_" + k], c, k))
                    add(0.5, lambda p=p, t=ptblk, c=cop:
                        emit_ctx_duo(p, 4, t, c))
                    add(0.5, lambda p=p, t=ptblk, c=cop:
                        emit_ctx_duo(p, 5, t, c))
                    if p == 1 and qw_i >= 1:
                        for c in range(4, 8):
                            add(0.5, lambda w=qw_i - 1, c=c:
                                outproj_one(w, c))
                    if last:
                        def last_partials(t=ptblk):
                            pl = lambda a, b: t[:, a:b, :]
                            stl = st["lo_l"]
                            nc.vector.tensor_add(pl(2, 4), pl(8, 10),
                                                 pl(10, 12))       # C: 8..11
                            nc.vector.tensor_add(pl(0, 2), pl(4, 6),
                                                 pl(2, 4))         # E+C
                            nc.vector.tensor_add(
                                stl[:, 0, :], t[:, 0, :],
                                t[:, 1, :])                        # kt0..11
                            nc.vector.tensor_add(
                                stl[:, 1, :], t[:, 12, :],
                                t[:, 13, :])                       # 12+13
                            nc.vector.tensor_add(
                                t[:, 0, :], stl[:, 0, :],
                                stl[:, 1, :])                      # S: 0..13
                            nc.vector.tensor_add(
                                t[:, 1, :], t[:, 0, :],
                                t[:, 14, :])                       # S': 0..14
                        add(0.1, last_partials)
                for half in range(2):
                    for kt_i in (4 * jj + 2 * half, 4 * jj + 2 * half + 1):
                        chunk(p, kt_i, qt_b, ptblk, f"s{blk_i}")
                    pops()
            prev = (p, qw_i, ptblk, cop, [6, 7], key)
            if p == 1 and qw_i + 1 < NQW:
                qt_cur = qt_nxt

        # ---------- tail ----------
        p_l, qw_l, ptblk_l, cop_l = prev[0], prev[1], prev[2], prev[3]
        dm = scp.tile([128, 2 * QW], F32, tag="sc", name="dm_warm")
        warm_mm = lambda: nc.tensor.matmul(
            dm[0:1, 0:64], onesK[:, 0:1], wmt[:, :], start=True, stop=True)
        # drain whatever is left (own duos 4,5 / po(w2) tail / den partials)
        emit_ctx_duo(p_l, 6, ptblk_l, cop_l)
        pops(budget=99.0)
        warm_mm()
        nc.vector.tensor_add(ptblk_l[:, 2, :], ptblk_l[:, 1, :],
                             ptblk_l[:, 15, :])          # acc = S' + kt15
        emit_ctx(p_l, 14, ptblk_l, cop_l)
        emit_ctx(p_l, 15, ptblk_l, cop_l)
        den_mm(ptblk_l, "rrc_l")
        warm_mm()
        rrc_l = st["rrc_l"]
        bc = pp.tile([128, QW], F32, tag="pp", name="bc_l")
        nc.tensor.matmul(bc[0:64, :], ones1[0:1, 0:64], rrc_l[0:1, 0:QW],
                         start=True, stop=True, tile_position=(0, 0))
        nc.tensor.matmul(bc[64:128, :], ones1[0:1, 0:64],
                         rrc_l[0:1, QW:2 * QW],
                         start=True, stop=True, tile_position=(0, 64))
        bcs = rrp.tile([128, QW], F32, tag="bcs", name="bcs_l")
        nc.vector.tensor_copy(bcs[:], bc[:])
        for tt in range(4):
            c0, c1 = tt * 128, (tt + 1) * 128
            nc.vector.tensor_mul(
                ctxP[p_l][:, qw_l * QW + c0:qw_l * QW + c1],
                cop_l[:, c0:c1], bcs[:, c0:c1])
            outproj_one(NQW - 1, 2 * tt, evq=nc.vector, dmaq=nc.sync)
            outproj_one(NQW - 1, 2 * tt + 1, evq=nc.scalar, dmaq=nc.scalar)

    nc.compile()
    return nc


def kernel(query, key, value, Wq, Wk, Wv, Wo):
    global _PROG, _LAST_IN_MAPS
    from concourse.bass_utils import run_bass_kernel_spmd

    if _PROG is None:
        _PROG = _build()
    nc = _PROG

    q2 = np.asarray(query, dtype=np.float32).reshape(B, S, D)
    k2 = np.asarray(key, dtype=np.float32).reshape(B, S, D)
    v2 = np.asarray(value, dtype=np.float32).reshape(B, S, D)
    Wq = np.asarray(Wq, dtype=np.float32)
    Wk = np.asarray(Wk, dtype=np.float32)
    Wv = np.asarray(Wv, dtype=np.float32)
    Wo = np.asarray(Wo, dtype=np.float32)

    xT = {}
    for b in range(B):
        xT[("q", b)] = np.ascontiguousarray(q2[b].T).astype(np.float16)
        xT[("k", b)] = np.ascontiguousarray(k2[b].T).astype(np.float16)
        xT[("v", b)] = np.ascontiguousarray(v2[b].T).astype(np.float16)

    in_maps = []
    for c in range(NCORES):
        b = c // 4
        l = c % 4
        rs = slice(CHD * l, CHD * (l + 1))
        in_maps.append({
            "xqT": xT[("q", b)],
            "xkT": xT[("k", b)],
            "xvT": xT[("v", b)],
            "wqT": (Wq[rs, :].T * SCALE).astype(np.float16),
            "wkT": Wk[rs, :].T.astype(np.float16),
            "wvT": Wv[rs, :].T.astype(np.float16),
            "woTs": np.ascontiguousarray(Wo[:, rs].T).astype(np.float16),
        })

    _LAST_IN_MAPS = in_maps
    res = run_bass_kernel_spmd(nc, in_maps, core_ids=list(range(NCORES)))
    parts = [res.results[c]["pout"].astype(np.float32) for c in range(NCORES)]
    out = np.empty((B, S, D), dtype=np.float32)
    for b in range(B):
        out[b] = parts[4 * b] + parts[4 * b + 1] + parts[4 * b + 2] + parts[4 * b + 3]
    return out


# revision 17
# speedup vs baseline: 1.1563x; 1.0171x over previous
"""Trainium2 Bass kernel for nn_MultiHeadAttention (B=2, S=2048, D=1024, H=16).

Sharding: 8 cores = 2 batch groups x 4 cores. Core c handles batch c//4 and
heads 4*(c%4) .. 4*(c%4)+4 (two head-pairs p=0,1). Each core computes Q/K/V
projections for its batch+heads, transposed-layout attention, and a partial
output projection over its 256 head-dims. Host sums the 4 partials per batch.

v3 design (vs v2 baseline @211.7us):
- ACT exp is the roofline (~143us busy): the schedule aims to start the exp
  stream ASAP and never starve it.
- 3 parallel DMA queues (sync/scalar HWDGE + gpsimd SWDGE); the ramp's
  critical Q/K loads split across queues (xq0 in two half-DMAs).
- all non-score PE work (projections, ctx, den matmuls, out-proj) is sliced
  into <~1us items in a FIFO insert queue, popped between score-chunk pairs
  with a per-point cost budget: the 2-deep sc PSUM ring only buffers 2.2us
  of ACT runway, so no insert slab may exceed it.
- den/bc broadcast matmuls in fp16 (were fp32 LOW_HIGH, 3x slower); the
  den->recip->bc chain is split across points so PE never waits on DVE.
- den tree adds C,D on the otherwise-idle gpsimd (DVE was co-bottleneck).
- tail: den pre-summed to one post-exp add, evictions split DVE/ACT,
  out-DMA split over both HWDGE queues, PE kept warm.
"""

import numpy as np

B, S, D, H = 2, 2048, 1024, 16
HD = D // H          # 64
NCORES = 8
HPC = 4              # heads per core
CHD = HPC * HD       # 256 head-dims per core
TOK = S              # tokens per core (one batch)
QW = 512             # query window
NQW = TOK // QW      # 4 windows
NKT = TOK // 128     # 16 key tiles
SCALE = 1.0 / np.sqrt(np.float32(D))  # 1/32, folded into Wq on host

_PROG = None
_LAST_IN_MAPS = None


def _build():
    from contextlib import ExitStack

    import concourse.bass as bass
    import concourse.tile as tile
    from concourse import bacc, mybir

    F16 = mybir.dt.float16
    F32 = mybir.dt.float32
    EXP = mybir.ActivationFunctionType.Exp

    nc = bacc.Bacc("TRN2", target_bir_lowering=False, debug=False,
                   num_devices=NCORES)

    xqT = nc.dram_tensor("xqT", [D, TOK], F16, kind="ExternalInput").ap()
    xkT = nc.dram_tensor("xkT", [D, TOK], F16, kind="ExternalInput").ap()
    xvT = nc.dram_tensor("xvT", [D, TOK], F16, kind="ExternalInput").ap()
    wqT = nc.dram_tensor("wqT", [D, CHD], F16, kind="ExternalInput").ap()
    wkT = nc.dram_tensor("wkT", [D, CHD], F16, kind="ExternalInput").ap()
    wvT = nc.dram_tensor("wvT", [D, CHD], F16, kind="ExternalInput").ap()
    woTs = nc.dram_tensor("woTs", [CHD, D], F16, kind="ExternalInput").ap()
    pout = nc.dram_tensor("pout", [TOK, D], F16, kind="ExternalOutput").ap()

    CB = 512            # V column-block width (tokens)

    with tile.TileContext(nc) as tc, ExitStack() as ctx:
        const = ctx.enter_context(tc.tile_pool(name="const", bufs=1))
        wq_sb = const.tile([128, 8, CHD], F16, tag="wq")
        wk_sb = const.tile([128, 8, CHD], F16, tag="wk")
        wv_sb = const.tile([128, 8, CHD], F16, tag="wv")
        wo_sb = [const.tile([128, D], F16, tag=f"wo{p}", name=f"wo{p}")
                 for p in range(2)]
        onesK = const.tile([128, 1], F16, tag="onesK")
        ones1 = const.tile([1, 128], F32, tag="ones1")

        nc.vector.memset(onesK, 1.0)
        nc.vector.memset(ones1, 1.0)

        warm = const.tile([1, 8], F32, tag="warm")
        nc.vector.memset(warm, 0.0)
        nc.scalar.activation(out=warm, in_=warm, func=EXP)
        wmt = const.tile([128, 64], F16, tag="wmt")
        nc.vector.memset(wmt, 1.0)

        big = ctx.enter_context(tc.tile_pool(name="big", bufs=1))
        KT = big.tile([128, 2, TOK], F16, tag="kt")          # [hd, pair, keys]
        vnat = big.tile([128, NKT, CHD], F16, tag="vnat")    # [keys, kt, hd]
        ctxP = [big.tile([128, TOK], F16, tag=f"ctxP{p}", name=f"ctxP{p}")
                for p in range(2)]

        xkb = ctx.enter_context(tc.tile_pool(name="xkb", bufs=3))
        xkb2 = ctx.enter_context(tc.tile_pool(name="xkb2", bufs=2))
        xvb = ctx.enter_context(tc.tile_pool(name="xvb", bufs=3))
        xqb = ctx.enter_context(tc.tile_pool(name="xqb", bufs=2))
        qtp = ctx.enter_context(tc.tile_pool(name="qtp", bufs=2))
        ptp = ctx.enter_context(tc.tile_pool(name="ptp", bufs=2))
        dtp = ctx.enter_context(tc.tile_pool(name="dtp", bufs=2))
        rrp = ctx.enter_context(tc.tile_pool(name="rrp", bufs=2))
        oev = ctx.enter_context(tc.tile_pool(name="oev", bufs=4))

        # PSUM (8 banks): sc ring 2x[128,1024] (4) + cop ring 2x[128,512] (2)
        # + shared proj/out/den/bcast pool 2x[128,512] (2)
        scp = ctx.enter_context(tc.tile_pool(name="scp", bufs=2, space="PSUM"))
        copp = ctx.enter_context(tc.tile_pool(name="copp", bufs=2, space="PSUM"))
        pp = ctx.enter_context(tc.tile_pool(name="pp", bufs=2, space="PSUM"))

        st = {}  # cross-item state (psum tiles, tree accumulators, rrc)

        # ---------- emission helpers ----------
        def dma_x_block(pool, src, c0, w, name, queue=None, halves=False):
            t = pool.tile([128, 8, w], F16, tag="xb", name=name)
            r = src[:, c0:c0 + w].rearrange("(ko ki) t -> ki ko t", ki=128)
            if halves:
                nc.sync.dma_start(out=t[:, 0:4, :], in_=r[:, 0:4, :])
                nc.scalar.dma_start(out=t[:, 4:8, :], in_=r[:, 4:8, :])
            else:
                (queue or nc.sync).dma_start(out=t, in_=r)
            return t

        def proj_q_quarter(qt, m, xq_t, phase, name):
            if phase == 0:
                st[name] = pp.tile([128, QW], F32, tag="pp", name=name)
            psq = st[name]
            for ko in range(4 * phase, 4 * phase + 4):
                nc.tensor.matmul(
                    psq[:], wq_sb[:, ko, m * 128:(m + 1) * 128],
                    xq_t[:, ko, :], start=(ko == 0), stop=(ko == 7))
            if phase == 1:
                nc.vector.tensor_copy(qt[:, m, :], psq[:])
                del st[name]

        def proj_k_half(c0, w, m, xk_t):
            psk = pp.tile([128, w], F32, tag="pp", name=f"psk{c0}_{m}")
            for ko in range(8):
                nc.tensor.matmul(
                    psk[:], wk_sb[:, ko, m * 128:(m + 1) * 128],
                    xk_t[:, ko, :], start=(ko == 0), stop=(ko == 7))
            nc.vector.tensor_copy(KT[:, m, c0:c0 + w], psk[:])

        def proj_v_slice(blk, half, xv_t, tt):
            name = f"pv{blk}_{half}"
            if tt == 0:
                st[name] = pp.tile([128, 2 * CHD], F32, tag="pp", name=name)
            pv = st[name]
            t0 = (2 * half + tt) * 128
            for ko in range(8):
                nc.tensor.matmul(
                    pv[:, tt * CHD:(tt + 1) * CHD],
                    xv_t[:, ko, t0:t0 + 128],
                    wv_sb[:, ko, :], start=(ko == 0), stop=(ko == 7))
            if tt == 1:
                nc.vector.tensor_copy(
                    vnat[:, 4 * blk + 2 * half:4 * blk + 2 * half + 2, :],
                    bass.AP(tensor=pv.tensor, offset=pv.offset,
                            ap=[list(pv.ap[0]), [CHD, 2], [1, CHD]]))
                del st[name]

        def chunk(p, kt_i, qt, ptblk, tag=""):
            k0 = kt_i * 128
            sc = scp.tile([128, 2 * QW], F32, tag="sc",
                          name=f"sc{tag}_{kt_i}")
            nc.tensor.matmul(
                sc[:, 0:QW], KT[0:64, p, k0:k0 + 128], qt[0:64, p, :],
                start=True, stop=True, tile_position=(0, 0))
            nc.tensor.matmul(
                sc[:, QW:2 * QW], KT[64:128, p, k0:k0 + 128], qt[64:128, p, :],
                start=True, stop=True, tile_position=(64, 0))
            nc.scalar.activation(out=ptblk[:, kt_i, :], in_=sc[:], func=EXP)

        def emit_ctx(p, kt_i, ptblk, cop):
            h0 = p * 128
            nc.tensor.matmul(
                cop[0:64, :], vnat[:, kt_i, h0:h0 + 64],
                ptblk[:, kt_i, 0:QW],
                start=(kt_i == 0), stop=(kt_i == NKT - 1),
                tile_position=(0, 0))
            nc.tensor.matmul(
                cop[64:128, :], vnat[:, kt_i, h0 + 64:h0 + 128],
                ptblk[:, kt_i, QW:2 * QW],
                start=(kt_i == 0), stop=(kt_i == NKT - 1),
                tile_position=(0, 64))

        def emit_ctx_duo(p, d, ptblk, cop):
            emit_ctx(p, 2 * d, ptblk, cop)
            emit_ctx(p, 2 * d + 1, ptblk, cop)

        def tree_lo(ptblk, key):
            stt = dtp.tile([128, 2, 2 * QW], F16, tag="dt", name=f"dt_{key}")
            pv = lambda a, b: ptblk[:, a:b, :]
            nc.vector.tensor_add(stt[:, 0:2, :], pv(0, 2), pv(2, 4))    # A
            nc.vector.tensor_add(pv(0, 2), pv(4, 6), pv(6, 8))          # B
            nc.vector.tensor_add(pv(4, 6), stt[:, 0:2, :], pv(0, 2))    # E
            st[key] = stt

        def tree_hi(ptblk, key):
            pv = lambda a, b: ptblk[:, a:b, :]
            nc.gpsimd.tensor_add(pv(2, 4), pv(8, 10), pv(10, 12))       # C
            nc.gpsimd.tensor_add(pv(6, 8), pv(12, 14), pv(14, 16))      # D
            nc.vector.tensor_add(pv(8, 10), pv(2, 4), pv(6, 8))         # F
            nc.vector.tensor_add(pv(0, 2), pv(4, 6), pv(8, 10))         # G
            nc.vector.tensor_add(ptblk[:, 2, :], ptblk[:, 0, :],
                                 ptblk[:, 1, :])                        # acc

        def den_mm(ptblk, key):
            """ones-matmul partition reduce per head, reciprocal into rrc."""
            accv = ptblk[:, 2, :]
            rrc = rrp.tile([1, 2 * QW], F32, tag="rrc", name=f"rrc_{key}")
            for h in range(2):
                den = pp.tile([128, QW], F32, tag="pp", name=f"den_{key}_{h}")
                nc.tensor.matmul(den[0:1, :], onesK[:, 0:1],
                                 accv[:, h * QW:(h + 1) * QW],
                                 start=True, stop=True)
                nc.vector.reciprocal_approx_fast(
                    rrc[0:1, h * QW:(h + 1) * QW], den[0:1, :])
            st[key] = rrc

        def den_bc(p, qw_i, rrc, cop, key):
            bc = pp.tile([128, QW], F32, tag="pp", name=f"bc_{key}")
            nc.tensor.matmul(bc[0:64, :], ones1[0:1, 0:64], rrc[0:1, 0:QW],
                             start=True, stop=True, tile_position=(0, 0))
            nc.tensor.matmul(bc[64:128, :], ones1[0:1, 0:64],
                             rrc[0:1, QW:2 * QW],
                             start=True, stop=True, tile_position=(0, 64))
            bcs = rrp.tile([128, QW], F32, tag="bcs", name=f"bcs_{key}")
            nc.vector.tensor_copy(bcs[:], bc[:])
            nc.vector.tensor_mul(
                ctxP[p][:, qw_i * QW:(qw_i + 1) * QW], cop[:], bcs[:])

        def outproj_one(qw_i, c, evq=None, dmaq=None):
            tt, et = c // 2, c % 2
            t0 = qw_i * QW + tt * 128
            po = pp.tile([128, 512], F32, tag="pp",
                         name=f"po{qw_i}_{tt}_{et}")
            for p in range(2):
                nc.tensor.matmul(
                    po[:], ctxP[p][:, t0:t0 + 128],
                    wo_sb[p][:, et * 512:(et + 1) * 512],
                    start=(p == 0), stop=(p == 1))
            ev = oev.tile([128, 512], F16, tag="oev")
            if evq is nc.scalar:
                nc.scalar.copy(ev[:], po[:])
            else:
                nc.vector.tensor_copy(ev[:], po[:])
            (dmaq or nc.sync).dma_start(
                out=pout[t0:t0 + 128, et * 512:(et + 1) * 512],
                in_=ev[:])

        # ---------- DMA issue ----------
        # sync:   xq0a, wk, xk[512:1024], xk[1536:2048], out-DMA w0..w2
        # scalar: xq0b, xk[0:256], xk[256:512], xk[1024:1536], xq1..3
        # gpsimd: wq, wv, xv0..2, wo0, wo1, xv3
        xq_t = [None] * NQW
        xq_t[0] = dma_x_block(xqb, xqT, 0, QW, "xq0", halves=True)
        nc.gpsimd.dma_start(
            out=wq_sb, in_=wqT.rearrange("(ko ki) m -> ki ko m", ki=128))
        # scalar (= the exp engine's queue) carries ONLY pre-stream loads:
        # a DMA_DIRECT2D's DGE time (1-13us under ring backpressure) blocks
        # every ACTIVATE behind it in the queue.
        KW = [256, 256, 512, 512, 512]
        KC0 = [0, 256, 512, 1024, 1536]
        KQ = [nc.scalar, nc.scalar, nc.sync, None, nc.sync]
        nc.sync.dma_start(
            out=wk_sb, in_=wkT.rearrange("(ko ki) m -> ki ko m", ki=128))
        xk_t = [dma_x_block(xkb if w == 512 else xkb2, xkT, c0, w,
                            f"xk{c0}", queue=q) if q is not None else None
                for c0, w, q in zip(KC0, KW, KQ)]

        # hold the PE clock gate open through the ramp's DMA shadow
        dmw = copp.tile([128, QW], F32, tag="cop", name="dm_ramp")
        def pe_warm(n):
            for _ in range(n):
                nc.tensor.matmul(dmw[0:1, 0:64], onesK[:, 0:1], wmt[:, :],
                                 start=True, stop=True)
        pe_warm(70)

        qt0 = qtp.tile([128, 2, QW], F16, tag="qt", name="qt0")
        qt1 = qtp.tile([128, 2, QW], F16, tag="qt", name="qt1")
        ptblk0 = ptp.tile([128, NKT, 2 * QW], F16, tag="pt", name="pt0")
        ptblk1 = ptp.tile([128, NKT, 2 * QW], F16, tag="pt", name="pt1")
        cop0 = copp.tile([128, QW], F32, tag="cop", name="cop0")
        cop1 = copp.tile([128, QW], F32, tag="cop", name="cop1")

        # ---------- ramp: block (0,p0), explicit schedule ----------
        C0 = lambda kt: chunk(0, kt, qt0, ptblk0, "r")
        proj_q_quarter(qt0, 0, xq_t[0], 0, "q0m0")
        proj_q_quarter(qt0, 0, xq_t[0], 1, "q0m0")
        # gpsimd-queue guard: V/wo loads (4.5MB) must not steal HBM
        # bandwidth from the ramp-critical Q/K loads (SDMA round-robins
        # across queues).  The tiny copy below waits for the qt0-m0
        # eviction, stalling the gpsimd DGE until the critical window ends.
        gscr = const.tile([128, 8], F16, tag="gscr")
        nc.gpsimd.tensor_copy(gscr[:], qt0[:, 0, 0:8])
        # gpsimd loads ordered by need-time (K feeds ACT directly; V/xq
        # lateness is absorbed by the insert queue)
        xk_t[3] = dma_x_block(xkb, xkT, KC0[3], KW[3], "xk1024",
                              queue=nc.gpsimd)
        nc.gpsimd.dma_start(
            out=wv_sb, in_=wvT.rearrange("(ko ki) m -> ki ko m", ki=128))
        xv_t = [dma_x_block(xvb, xvT, b * CB, CB, f"xv{b}", queue=nc.gpsimd)
                for b in range(2)]
        xq_t[1] = dma_x_block(xqb, xqT, QW, QW, "xq1", queue=nc.gpsimd)
        xv_t.append(dma_x_block(xvb, xvT, 2 * CB, CB, "xv2",
                                queue=nc.gpsimd))
        xv_t.append(dma_x_block(xvb, xvT, 3 * CB, CB, "xv3",
                                queue=nc.gpsimd))
        nc.gpsimd.dma_start(out=wo_sb[0], in_=woTs[0:128, :])
        nc.gpsimd.dma_start(out=wo_sb[1], in_=woTs[128:256, :])
        xq_t[2] = dma_x_block(xqb, xqT, 2 * QW, QW, "xq2", queue=nc.gpsimd)
        xq_t[3] = dma_x_block(xqb, xqT, 3 * QW, QW, "xq3", queue=nc.gpsimd)
        pe_warm(25)
        proj_k_half(0, 256, 0, xk_t[0])
        C0(0)
        proj_q_quarter(qt0, 1, xq_t[0], 0, "q0m1")
        C0(1)
        proj_q_quarter(qt0, 1, xq_t[0], 1, "q0m1")
        proj_k_half(256, 256, 0, xk_t[1])
        C0(2)
        proj_k_half(0, 256, 1, xk_t[0])
        C0(3)
        proj_k_half(512, 512, 0, xk_t[2])
        C0(4)
        proj_k_half(256, 256, 1, xk_t[1])
        C0(5)
        C0(6)
        proj_k_half(1024, 512, 0, xk_t[3])
        C0(7)
        C0(8)
        proj_v_slice(0, 0, xv_t[0], 0)
        C0(9)
        proj_v_slice(0, 0, xv_t[0], 1)
        proj_k_half(512, 512, 1, xk_t[2])
        C0(10)
        proj_v_slice(0, 1, xv_t[0], 0)
        C0(11)
        proj_v_slice(0, 1, xv_t[0], 1)
        proj_k_half(1536, 512, 0, xk_t[4])
        C0(12)
        C0(13)
        proj_k_half(1024, 512, 1, xk_t[3])
        C0(14)
        C0(15)

        # ---------- insert queue ----------
        queue = []  # (cost, fn)

        def pops(budget=1.35):
            spent = 0.0
            while queue and spent + queue[0][0] <= budget:
                cost, fn = queue.pop(0)
                fn()
                spent += cost

        def add(cost, fn):
            queue.append((cost, fn))

        # remaining ramp leftovers -> head of queue for the fill era
        add(0.9, lambda: proj_v_slice(1, 0, xv_t[1], 0))
        add(0.9, lambda: proj_v_slice(1, 0, xv_t[1], 1))
        add(0.9, lambda: proj_k_half(1536, 512, 1, xk_t[4]))
        add(0.9, lambda: proj_v_slice(1, 1, xv_t[1], 0))
        add(0.9, lambda: proj_v_slice(1, 1, xv_t[1], 1))

        # ---------- fill era: block (0,p1) ----------
        # items: remaining V, ctx(0,p0), den(0,p0), qt1-m0.  The den tree
        # writes pt slots 0,1,4,5 (lo) and 2,3,6..9 (hi): it must trail the
        # ctx duos that read those slots (d0,d2 before lo; d1,d3,d4 before
        # hi) -- FIFO pop order preserves this.
        add(0.5, lambda: emit_ctx_duo(0, 0, ptblk0, cop0))
        add(0.5, lambda: emit_ctx_duo(0, 1, ptblk0, cop0))
        add(0.9, lambda: proj_v_slice(2, 0, xv_t[2], 0))
        add(0.9, lambda: proj_v_slice(2, 0, xv_t[2], 1))
        add(0.5, lambda: emit_ctx_duo(0, 2, ptblk0, cop0))
        add(0.1, lambda: tree_lo(ptblk0, "lo_00"))
        add(0.9, lambda: proj_v_slice(2, 1, xv_t[2], 0))
        add(0.9, lambda: proj_v_slice(2, 1, xv_t[2], 1))
        add(0.5, lambda: emit_ctx_duo(0, 3, ptblk0, cop0))
        add(0.5, lambda: emit_ctx_duo(0, 4, ptblk0, cop0))
        add(0.1, lambda: tree_hi(ptblk0, "lo_00"))
        add(0.9, lambda: proj_v_slice(3, 0, xv_t[3], 0))
        add(0.9, lambda: proj_v_slice(3, 0, xv_t[3], 1))
        add(0.5, lambda: emit_ctx_duo(0, 5, ptblk0, cop0))
        add(0.15, lambda: den_mm(ptblk0, "rrc_00"))
        add(0.9, lambda: proj_v_slice(3, 1, xv_t[3], 0))
        add(0.9, lambda: proj_v_slice(3, 1, xv_t[3], 1))
        add(0.5, lambda: emit_ctx_duo(0, 6, ptblk0, cop0))
        add(0.5, lambda: emit_ctx_duo(0, 7, ptblk0, cop0))
        add(0.15, lambda: den_bc(0, 0, st["rrc_00"], cop0, "00"))
        add(0.85, lambda: proj_q_quarter(qt1, 0, xq_t[1], 0, "q1m0"))
        add(0.85, lambda: proj_q_quarter(qt1, 0, xq_t[1], 1, "q1m0"))

        for kt_i in range(NKT):
            chunk(1, kt_i, qt0, ptblk1, "f")
            pops()

        # ---------- steady blocks ----------
        blocks = [(qw_i, p) for qw_i in range(NQW) for p in range(2)][2:]
        prev = (1, 0, ptblk1, cop1, list(range(8)), "01")  # p,qw,pt,cop,duos
        qt_cur = qt1
        qt_nxt = None
        for bi, (qw_i, p) in enumerate(blocks):
            blk_i = bi + 2
            last = bi == len(blocks) - 1
            key = f"{qw_i}{p}"
            ptblk = ptp.tile([128, NKT, 2 * QW], F16, tag="pt",
                             name=f"pt{blk_i}")
            cop = copp.tile([128, QW], F32, tag="cop", name=f"cop{blk_i}")
            qt_b = qt_cur
            pp_, pqw, ppt, pcop, pduos, pkey = prev
            if p == 1 and qw_i + 1 < NQW:
                qt_nxt = qtp.tile([128, 2, QW], F16, tag="qt",
                                  name=f"qt{qw_i + 1}")
            # force-drain before this block's chunks: leftover items may
            # include this block's qt producers (PE in-order: a score that
            # waits on a later-emitted PE producer would deadlock)
            pops(budget=99.0)
            for jj in range(4):
                if jj == 0:
                    for d in pduos:
                        add(0.5, lambda a=pp_, d=d, t=ppt, c=pcop:
                            emit_ctx_duo(a, d, t, c))
                    if bi == 0:
                        # qt1-m1 for the sibling block (1,p1)
                        add(0.85, lambda: proj_q_quarter(
                            qt1, 1, xq_t[1], 0, "q1m1"))
                        add(0.85, lambda: proj_q_quarter(
                            qt1, 1, xq_t[1], 1, "q1m1"))
                    if p == 1 and qw_i + 1 < NQW:
                        nq = qw_i + 1
                        add(0.85, lambda nq=nq, qn=qt_nxt: proj_q_quarter(
                            qn, 0, xq_t[nq], 0, f"q{nq}m0"))
                        add(0.85, lambda nq=nq, qn=qt_nxt: proj_q_quarter(
                            qn, 0, xq_t[nq], 1, f"q{nq}m0"))
                if jj == 1:
                    add(0.1, lambda t=ppt, k=pkey: tree_lo(t, "lo_" + k))
                    add(0.1, lambda t=ppt, k=pkey: tree_hi(t, "lo_" + k))
                    add(0.5, lambda p=p, t=ptblk, c=cop:
                        emit_ctx_duo(p, 0, t, c))
                    add(0.5, lambda p=p, t=ptblk, c=cop:
                        emit_ctx_duo(p, 1, t, c))
                    if p == 1 and qw_i + 1 < NQW:
                        nq = qw_i + 1
                        add(0.85, lambda nq=nq, qn=qt_nxt: proj_q_quarter(
                            qn, 1, xq_t[nq], 0, f"q{nq}m1"))
                        add(0.85, lambda nq=nq, qn=qt_nxt: proj_q_quarter(
                            qn, 1, xq_t[nq], 1, f"q{nq}m1"))
                if jj == 2:
                    add(0.15, lambda t=ppt, k=pkey: den_mm(t, "rrc_" + k))
                    add(0.5, lambda p=p, t=ptblk, c=cop:
                        emit_ctx_duo(p, 2, t, c))
                    add(0.5, lambda p=p, t=ptblk, c=cop:
                        emit_ctx_duo(p, 3, t, c))
                    if p == 1 and qw_i >= 1:
                        for c in range(4):
                            add(0.5, lambda w=qw_i - 1, c=c:
                                outproj_one(w, c))
                    if last:
                        add(0.1, lambda t=ptblk: tree_lo(t, "lo_l"))
                if jj == 3:
                    add(0.15, lambda a=pp_, w=pqw, c=pcop, k=pkey:
                        den_bc(a, w, st["rrc_" + k], c, k))
                    add(0.5, lambda p=p, t=ptblk, c=cop:
                        emit_ctx_duo(p, 4, t, c))
                    add(0.5, lambda p=p, t=ptblk, c=cop:
                        emit_ctx_duo(p, 5, t, c))
                    if p == 1 and qw_i >= 1:
                        for c in range(4, 8):
                            add(0.5, lambda w=qw_i - 1, c=c:
                                outproj_one(w, c))
                    if last:
                        def last_partials(t=ptblk):
                            pl = lambda a, b: t[:, a:b, :]
                            stl = st["lo_l"]
                            nc.vector.tensor_add(pl(2, 4), pl(8, 10),
                                                 pl(10, 12))       # C: 8..11
                            nc.vector.tensor_add(pl(0, 2), pl(4, 6),
                                                 pl(2, 4))         # E+C
                            nc.vector.tensor_add(
                                stl[:, 0, :], t[:, 0, :],
                                t[:, 1, :])                        # kt0..11
                            nc.vector.tensor_add(
                                stl[:, 1, :], t[:, 12, :],
                                t[:, 13, :])                       # 12+13
                            nc.vector.tensor_add(
                                t[:, 0, :], stl[:, 0, :],
                                stl[:, 1, :])                      # S: 0..13
                            nc.vector.tensor_add(
                                t[:, 1, :], t[:, 0, :],
                                t[:, 14, :])                       # S': 0..14
                        add(0.1, last_partials)
                for half in range(2):
                    for kt_i in (4 * jj + 2 * half, 4 * jj + 2 * half + 1):
                        chunk(p, kt_i, qt_b, ptblk, f"s{blk_i}")
                    pops()
            prev = (p, qw_i, ptblk, cop, [6, 7], key)
            if p == 1 and qw_i + 1 < NQW:
                qt_cur = qt_nxt

        # ---------- tail ----------
        p_l, qw_l, ptblk_l, cop_l = prev[0], prev[1], prev[2], prev[3]
        dm = scp.tile([128, 2 * QW], F32, tag="sc", name="dm_warm")
        warm_mm = lambda: nc.tensor.matmul(
            dm[0:1, 0:64], onesK[:, 0:1], wmt[:, :], start=True, stop=True)
        # drain whatever is left (own duos 4,5 / po(w2) tail / den partials)
        emit_ctx_duo(p_l, 6, ptblk_l, cop_l)
        pops(budget=99.0)
        warm_mm()
        nc.vector.tensor_add(ptblk_l[:, 2, :], ptblk_l[:, 1, :],
                             ptblk_l[:, 15, :])          # acc = S' + kt15
        emit_ctx(p_l, 14, ptblk_l, cop_l)
        emit_ctx(p_l, 15, ptblk_l, cop_l)
        den_mm(ptblk_l, "rrc_l")
        warm_mm()
        rrc_l = st["rrc_l"]
        bc = pp.tile([128, QW], F32, tag="pp", name="bc_l")
        nc.tensor.matmul(bc[0:64, :], ones1[0:1, 0:64], rrc_l[0:1, 0:QW],
                         start=True, stop=True, tile_position=(0, 0))
        nc.tensor.matmul(bc[64:128, :], ones1[0:1, 0:64],
                         rrc_l[0:1, QW:2 * QW],
                         start=True, stop=True, tile_position=(0, 64))
        bcs = rrp.tile([128, QW], F32, tag="bcs", name="bcs_l")
        nc.vector.tensor_copy(bcs[:], bc[:])
        for tt in range(4):
            c0, c1 = tt * 128, (tt + 1) * 128
            nc.vector.tensor_mul(
                ctxP[p_l][:, qw_l * QW + c0:qw_l * QW + c1],
                cop_l[:, c0:c1], bcs[:, c0:c1])
            outproj_one(NQW - 1, 2 * tt, evq=nc.vector, dmaq=nc.sync)
            outproj_one(NQW - 1, 2 * tt + 1, evq=nc.scalar, dmaq=nc.scalar)

    nc.compile()
    return nc


def kernel(query, key, value, Wq, Wk, Wv, Wo):
    global _PROG, _LAST_IN_MAPS
    from concourse.bass_utils import run_bass_kernel_spmd

    if _PROG is None:
        _PROG = _build()
    nc = _PROG

    q2 = np.asarray(query, dtype=np.float32).reshape(B, S, D)
    k2 = np.asarray(key, dtype=np.float32).reshape(B, S, D)
    v2 = np.asarray(value, dtype=np.float32).reshape(B, S, D)
    Wq = np.asarray(Wq, dtype=np.float32)
    Wk = np.asarray(Wk, dtype=np.float32)
    Wv = np.asarray(Wv, dtype=np.float32)
    Wo = np.asarray(Wo, dtype=np.float32)

    xT = {}
    for b in range(B):
        xT[("q", b)] = np.ascontiguousarray(q2[b].T).astype(np.float16)
        xT[("k", b)] = np.ascontiguousarray(k2[b].T).astype(np.float16)
        xT[("v", b)] = np.ascontiguousarray(v2[b].T).astype(np.float16)

    in_maps = []
    for c in range(NCORES):
        b = c // 4
        l = c % 4
        rs = slice(CHD * l, CHD * (l + 1))
        in_maps.append({
            "xqT": xT[("q", b)],
            "xkT": xT[("k", b)],
            "xvT": xT[("v", b)],
            "wqT": (Wq[rs, :].T * SCALE).astype(np.float16),
            "wkT": Wk[rs, :].T.astype(np.float16),
            "wvT": Wv[rs, :].T.astype(np.float16),
            "woTs": np.ascontiguousarray(Wo[:, rs].T).astype(np.float16),
        })

    _LAST_IN_MAPS = in_maps
    res = run_bass_kernel_spmd(nc, in_maps, core_ids=list(range(NCORES)))
    parts = [res.results[c]["pout"].astype(np.float32) for c in range(NCORES)]
    out = np.empty((B, S, D), dtype=np.float32)
    for b in range(B):
        out[b] = parts[4 * b] + parts[4 * b + 1] + parts[4 * b + 2] + parts[4 * b + 3]
    return out


# revision 22
# speedup vs baseline: 1.3796x; 1.1931x over previous
"""Trainium2 Bass kernel for nn_MultiHeadAttention (B=2, S=2048, D=1024, H=16).

Sharding: 8 cores = 2 batch groups x 4 cores. Core c handles batch c//4 and
heads 4*(c%4) .. 4*(c%4)+4 (two head-pairs p=0,1). Each core computes Q/K/V
projections for its batch+heads, transposed-layout attention, and a partial
output projection over its 256 head-dims. Host sums the 4 partials per batch.

v3 design (vs v2 baseline @211.7us):
- ACT exp is the roofline (~143us busy): the schedule aims to start the exp
  stream ASAP and never starve it.
- 3 parallel DMA queues (sync/scalar HWDGE + gpsimd SWDGE); the ramp's
  critical Q/K loads split across queues (xq0 in two half-DMAs).
- all non-score PE work (projections, ctx, den matmuls, out-proj) is sliced
  into <~1us items in a FIFO insert queue, popped between score-chunk pairs
  with a per-point cost budget: the 2-deep sc PSUM ring only buffers 2.2us
  of ACT runway, so no insert slab may exceed it.
- den/bc broadcast matmuls in fp16 (were fp32 LOW_HIGH, 3x slower); the
  den->recip->bc chain is split across points so PE never waits on DVE.
- den tree adds C,D on the otherwise-idle gpsimd (DVE was co-bottleneck).
- tail: den pre-summed to one post-exp add, evictions split DVE/ACT,
  out-DMA split over both HWDGE queues, PE kept warm.
"""

import numpy as np

B, S, D, H = 2, 2048, 1024, 16
HD = D // H          # 64
NCORES = 8
HPC = 4              # heads per core
CHD = HPC * HD       # 256 head-dims per core
TOK = S              # tokens per core (one batch)
QW = 512             # query window
NQW = TOK // QW      # 4 windows
NKT = TOK // 128     # 16 key tiles
SCALE = 1.0 / np.sqrt(np.float32(D))  # 1/32, folded into Wq on host

_PROG = None
_LAST_IN_MAPS = None


def _build():
    from contextlib import ExitStack

    import concourse.bass as bass
    import concourse.tile as tile
    from concourse import bacc, mybir

    F16 = mybir.dt.float16
    F32 = mybir.dt.float32
    EXP = mybir.ActivationFunctionType.Exp

    nc = bacc.Bacc("TRN2", target_bir_lowering=False, debug=False,
                   num_devices=NCORES)

    xqT = nc.dram_tensor("xqT", [D, TOK], F16, kind="ExternalInput").ap()
    xkT = nc.dram_tensor("xkT", [D, TOK], F16, kind="ExternalInput").ap()
    xvT = nc.dram_tensor("xvT", [D, TOK], F16, kind="ExternalInput").ap()
    wqT = nc.dram_tensor("wqT", [D, CHD], F16, kind="ExternalInput").ap()
    wkT = nc.dram_tensor("wkT", [D, CHD], F16, kind="ExternalInput").ap()
    wvT = nc.dram_tensor("wvT", [D, CHD], F16, kind="ExternalInput").ap()
    woTs = nc.dram_tensor("woTs", [CHD, D], F16, kind="ExternalInput").ap()
    pout = nc.dram_tensor("pout", [TOK, D], F16, kind="ExternalOutput").ap()

    CB = 512            # V column-block width (tokens)

    with tile.TileContext(nc) as tc, ExitStack() as ctx:
        const = ctx.enter_context(tc.tile_pool(name="const", bufs=1))
        wq_sb = const.tile([128, 8, CHD], F16, tag="wq")
        wk_sb = const.tile([128, 8, CHD], F16, tag="wk")
        wv_sb = const.tile([128, 8, CHD], F16, tag="wv")
        wo_sb = [const.tile([128, D], F16, tag=f"wo{p}", name=f"wo{p}")
                 for p in range(2)]
        onesK = const.tile([128, 1], F16, tag="onesK")
        ones1 = const.tile([1, 128], F32, tag="ones1")

        nc.vector.memset(onesK, 1.0)
        nc.vector.memset(ones1, 1.0)

        warm = const.tile([1, 8], F32, tag="warm")
        nc.vector.memset(warm, 0.0)
        nc.scalar.activation(out=warm, in_=warm, func=EXP)
        wmt = const.tile([128, 64], F16, tag="wmt")
        nc.vector.memset(wmt, 1.0)

        big = ctx.enter_context(tc.tile_pool(name="big", bufs=1))
        KT = big.tile([128, 2, TOK], F16, tag="kt")          # [hd, pair, keys]
        vnat = big.tile([128, NKT, CHD], F16, tag="vnat")    # [keys, kt, hd]
        ctxP = [big.tile([128, TOK], F16, tag=f"ctxP{p}", name=f"ctxP{p}")
                for p in range(2)]

        xkb = ctx.enter_context(tc.tile_pool(name="xkb", bufs=3))
        xkb2 = ctx.enter_context(tc.tile_pool(name="xkb2", bufs=2))
        xvb = ctx.enter_context(tc.tile_pool(name="xvb", bufs=3))
        xqb = ctx.enter_context(tc.tile_pool(name="xqb", bufs=2))
        qtp = ctx.enter_context(tc.tile_pool(name="qtp", bufs=2))
        ptp = ctx.enter_context(tc.tile_pool(name="ptp", bufs=2))
        dtp = ctx.enter_context(tc.tile_pool(name="dtp", bufs=2))
        rrp = ctx.enter_context(tc.tile_pool(name="rrp", bufs=2))
        oev = ctx.enter_context(tc.tile_pool(name="oev", bufs=4))

        # PSUM (8 banks): sc ring 2x[128,1024] (4) + cop ring 2x[128,512] (2)
        # + shared proj/out/den/bcast pool 2x[128,512] (2)
        scp = ctx.enter_context(tc.tile_pool(name="scp", bufs=2, space="PSUM"))
        copp = ctx.enter_context(tc.tile_pool(name="copp", bufs=2, space="PSUM"))
        pp = ctx.enter_context(tc.tile_pool(name="pp", bufs=2, space="PSUM"))

        st = {}  # cross-item state (psum tiles, tree accumulators, rrc)

        # ---------- emission helpers ----------
        def dma_x_block(pool, src, c0, w, name, queue=None, halves=False,
                        gate=None):
            t = pool.tile([128, 8, w], F16, tag="xb", name=name)
            r = src[:, c0:c0 + w].rearrange("(ko ki) t -> ki ko t", ki=128)
            if gate is not None:
                # Tile's scheduler hoists ready DMAs into idle DGE slots
                # regardless of emission order; a tiny write into the DMA
                # target (reading a late tile) gives the DMA a real WAW
                # dependency so non-critical loads don't steal HBM bandwidth
                # from the ramp-critical ones.
                nc.gpsimd.tensor_copy(t[:, 0, 0:8], gate)
            if halves:
                nc.sync.dma_start(out=t[:, 0:4, :], in_=r[:, 0:4, :])
                nc.scalar.dma_start(out=t[:, 4:8, :], in_=r[:, 4:8, :])
            else:
                (queue or nc.sync).dma_start(out=t, in_=r)
            return t

        def proj_q_quarter(qt, m, xq_t, phase, name):
            if phase == 0:
                st[name] = pp.tile([128, QW], F32, tag="pp", name=name)
            psq = st[name]
            for ko in range(4 * phase, 4 * phase + 4):
                nc.tensor.matmul(
                    psq[:], wq_sb[:, ko, m * 128:(m + 1) * 128],
                    xq_t[:, ko, :], start=(ko == 0), stop=(ko == 7))
            if phase == 1:
                nc.vector.tensor_copy(qt[:, m, :], psq[:])
                del st[name]

        def proj_k_half(c0, w, m, xk_t):
            psk = pp.tile([128, w], F32, tag="pp", name=f"psk{c0}_{m}")
            for ko in range(8):
                nc.tensor.matmul(
                    psk[:], wk_sb[:, ko, m * 128:(m + 1) * 128],
                    xk_t[:, ko, :], start=(ko == 0), stop=(ko == 7))
            nc.vector.tensor_copy(KT[:, m, c0:c0 + w], psk[:])

        def proj_v_slice(blk, half, xv_t, tt):
            name = f"pv{blk}_{half}"
            if tt == 0:
                st[name] = pp.tile([128, 2 * CHD], F32, tag="pp", name=name)
            pv = st[name]
            t0 = (2 * half + tt) * 128
            for ko in range(8):
                nc.tensor.matmul(
                    pv[:, tt * CHD:(tt + 1) * CHD],
                    xv_t[:, ko, t0:t0 + 128],
                    wv_sb[:, ko, :], start=(ko == 0), stop=(ko == 7))
            if tt == 1:
                nc.vector.tensor_copy(
                    vnat[:, 4 * blk + 2 * half:4 * blk + 2 * half + 2, :],
                    bass.AP(tensor=pv.tensor, offset=pv.offset,
                            ap=[list(pv.ap[0]), [CHD, 2], [1, CHD]]))
                del st[name]

        def chunk(p, kt_i, qt, ptblk, tag=""):
            k0 = kt_i * 128
            sc = scp.tile([128, 2 * QW], F32, tag="sc",
                          name=f"sc{tag}_{kt_i}")
            nc.tensor.matmul(
                sc[:, 0:QW], KT[0:64, p, k0:k0 + 128], qt[0:64, p, :],
                start=True, stop=True, tile_position=(0, 0))
            nc.tensor.matmul(
                sc[:, QW:2 * QW], KT[64:128, p, k0:k0 + 128], qt[64:128, p, :],
                start=True, stop=True, tile_position=(64, 0))
            nc.scalar.activation(out=ptblk[:, kt_i, :], in_=sc[:], func=EXP)

        def emit_ctx(p, kt_i, ptblk, cop):
            h0 = p * 128
            nc.tensor.matmul(
                cop[0:64, :], vnat[:, kt_i, h0:h0 + 64],
                ptblk[:, kt_i, 0:QW],
                start=(kt_i == 0), stop=(kt_i == NKT - 1),
                tile_position=(0, 0))
            nc.tensor.matmul(
                cop[64:128, :], vnat[:, kt_i, h0 + 64:h0 + 128],
                ptblk[:, kt_i, QW:2 * QW],
                start=(kt_i == 0), stop=(kt_i == NKT - 1),
                tile_position=(0, 64))

        def emit_ctx_duo(p, d, ptblk, cop):
            emit_ctx(p, 2 * d, ptblk, cop)
            emit_ctx(p, 2 * d + 1, ptblk, cop)

        def tree_lo(ptblk, key):
            stt = dtp.tile([128, 2, 2 * QW], F16, tag="dt", name=f"dt_{key}")
            pv = lambda a, b: ptblk[:, a:b, :]
            nc.vector.tensor_add(stt[:, 0:2, :], pv(0, 2), pv(2, 4))    # A
            nc.vector.tensor_add(pv(0, 2), pv(4, 6), pv(6, 8))          # B
            nc.vector.tensor_add(pv(4, 6), stt[:, 0:2, :], pv(0, 2))    # E
            st[key] = stt

        def tree_hi(ptblk, key):
            # all-DVE: gpsimd adds are ~4x the latency and sit on the
            # ptp-ring critical path (exp of block i+2 waits on them)
            pv = lambda a, b: ptblk[:, a:b, :]
            nc.vector.tensor_add(pv(2, 4), pv(8, 10), pv(10, 12))       # C
            nc.vector.tensor_add(pv(6, 8), pv(12, 14), pv(14, 16))      # D
            nc.vector.tensor_add(pv(8, 10), pv(2, 4), pv(6, 8))         # F
            nc.vector.tensor_add(pv(0, 2), pv(4, 6), pv(8, 10))         # G
            nc.vector.tensor_add(ptblk[:, 2, :], ptblk[:, 0, :],
                                 ptblk[:, 1, :])                        # acc

        def den_mm(ptblk, key):
            """ones-matmul partition reduce per head, reciprocal into rrc."""
            accv = ptblk[:, 2, :]
            rrc = rrp.tile([1, 2 * QW], F32, tag="rrc", name=f"rrc_{key}")
            for h in range(2):
                den = pp.tile([128, QW], F32, tag="pp", name=f"den_{key}_{h}")
                nc.tensor.matmul(den[0:1, :], onesK[:, 0:1],
                                 accv[:, h * QW:(h + 1) * QW],
                                 start=True, stop=True)
                nc.vector.reciprocal_approx_fast(
                    rrc[0:1, h * QW:(h + 1) * QW], den[0:1, :])
            st[key] = rrc

        def den_bc(p, qw_i, rrc, cop, key):
            bc = pp.tile([128, QW], F32, tag="pp", name=f"bc_{key}")
            nc.tensor.matmul(bc[0:64, :], ones1[0:1, 0:64], rrc[0:1, 0:QW],
                             start=True, stop=True, tile_position=(0, 0))
            nc.tensor.matmul(bc[64:128, :], ones1[0:1, 0:64],
                             rrc[0:1, QW:2 * QW],
                             start=True, stop=True, tile_position=(0, 64))
            bcs = rrp.tile([128, QW], F32, tag="bcs", name=f"bcs_{key}")
            nc.vector.tensor_copy(bcs[:], bc[:])
            nc.vector.tensor_mul(
                ctxP[p][:, qw_i * QW:(qw_i + 1) * QW], cop[:], bcs[:])

        def outproj_one(qw_i, c, evq=None, dmaq=None):
            tt, et = c // 2, c % 2
            t0 = qw_i * QW + tt * 128
            po = pp.tile([128, 512], F32, tag="pp",
                         name=f"po{qw_i}_{tt}_{et}")
            for p in range(2):
                nc.tensor.matmul(
                    po[:], ctxP[p][:, t0:t0 + 128],
                    wo_sb[p][:, et * 512:(et + 1) * 512],
                    start=(p == 0), stop=(p == 1))
            ev = oev.tile([128, 512], F16, tag="oev")
            if evq is nc.scalar:
                nc.scalar.copy(ev[:], po[:])
            else:
                nc.vector.tensor_copy(ev[:], po[:])
            (dmaq or nc.sync).dma_start(
                out=pout[t0:t0 + 128, et * 512:(et + 1) * 512],
                in_=ev[:])

        # ---------- DMA issue ----------
        # sync:   xq0a, wk, xk[512:1024], xk[1536:2048], out-DMA w0..w2
        # scalar: xq0b, xk[0:256], xk[256:512], xk[1024:1536], xq1..3
        # gpsimd: wq, wv, xv0..2, wo0, wo1, xv3
        xq_t = [None] * NQW
        xq_t[0] = dma_x_block(xqb, xqT, 0, QW, "xq0", halves=True)
        nc.gpsimd.dma_start(
            out=wq_sb, in_=wqT.rearrange("(ko ki) m -> ki ko m", ki=128))
        # scalar (= the exp engine's queue) carries ONLY pre-stream loads:
        # a DMA_DIRECT2D's DGE time (1-13us under ring backpressure) blocks
        # every ACTIVATE behind it in the queue.
        KW = [256, 256, 512, 512, 512]
        KC0 = [0, 256, 512, 1024, 1536]
        KQ = [nc.scalar, nc.scalar, nc.sync, None, nc.sync]
        nc.sync.dma_start(
            out=wk_sb, in_=wkT.rearrange("(ko ki) m -> ki ko m", ki=128))
        xk_t = [dma_x_block(xkb if w == 512 else xkb2, xkT, c0, w,
                            f"xk{c0}", queue=q) if q is not None else None
                for c0, w, q in zip(KC0, KW, KQ)]

        # hold the PE clock gate open through the ramp's DMA shadow
        dmw = copp.tile([128, QW], F32, tag="cop", name="dm_ramp")
        def pe_warm(n):
            for _ in range(n):
                nc.tensor.matmul(dmw[0:1, 0:64], onesK[:, 0:1], wmt[:, :],
                                 start=True, stop=True)
        pe_warm(70)

        qt0 = qtp.tile([128, 2, QW], F16, tag="qt", name="qt0")
        qt1 = qtp.tile([128, 2, QW], F16, tag="qt", name="qt1")
        ptblk0 = ptp.tile([128, NKT, 2 * QW], F16, tag="pt", name="pt0")
        ptblk1 = ptp.tile([128, NKT, 2 * QW], F16, tag="pt", name="pt1")
        cop0 = copp.tile([128, QW], F32, tag="cop", name="cop0")
        cop1 = copp.tile([128, QW], F32, tag="cop", name="cop1")

        # ---------- ramp: block (0,p0), explicit schedule ----------
        C0 = lambda kt: chunk(0, kt, qt0, ptblk0, "r")
        proj_q_quarter(qt0, 0, xq_t[0], 0, "q0m0")
        proj_q_quarter(qt0, 0, xq_t[0], 1, "q0m0")
        # gpsimd-queue guard: V/wo loads (4.5MB) must not steal HBM
        # bandwidth from the ramp-critical Q/K loads (SDMA round-robins
        # across queues).  The tiny copy below waits for the qt0-m0
        # eviction, stalling the gpsimd DGE until the critical window ends.
        # gpsimd loads ordered by need-time, all gated on the qt0-m0 evict
        # (K feeds ACT directly; V/xq lateness is absorbed by the queue)
        gt = qt0[:, 0, 0:8]
        xk_t[3] = dma_x_block(xkb, xkT, KC0[3], KW[3], "xk1024",
                              queue=nc.gpsimd, gate=gt)
        nc.gpsimd.tensor_copy(wv_sb[:, 0, 0:8], gt)
        nc.gpsimd.dma_start(
            out=wv_sb, in_=wvT.rearrange("(ko ki) m -> ki ko m", ki=128))
        xv_t = [dma_x_block(xvb, xvT, b * CB, CB, f"xv{b}",
                            queue=nc.gpsimd, gate=gt)
                for b in range(2)]
        xq_t[1] = dma_x_block(xqb, xqT, QW, QW, "xq1", queue=nc.gpsimd,
                              gate=gt)
        xv_t.append(dma_x_block(xvb, xvT, 2 * CB, CB, "xv2",
                                queue=nc.gpsimd, gate=gt))
        xv_t.append(dma_x_block(xvb, xvT, 3 * CB, CB, "xv3",
                                queue=nc.gpsimd, gate=gt))
        nc.gpsimd.tensor_copy(wo_sb[0][:, 0:8], gt)
        nc.gpsimd.dma_start(out=wo_sb[0], in_=woTs[0:128, :])
        nc.gpsimd.tensor_copy(wo_sb[1][:, 0:8], gt)
        nc.gpsimd.dma_start(out=wo_sb[1], in_=woTs[128:256, :])
        xq_t[2] = dma_x_block(xqb, xqT, 2 * QW, QW, "xq2", queue=nc.gpsimd,
                              gate=gt)
        xq_t[3] = dma_x_block(xqb, xqT, 3 * QW, QW, "xq3", queue=nc.gpsimd,
                              gate=gt)
        pe_warm(25)
        proj_k_half(0, 256, 0, xk_t[0])
        C0(0)
        proj_q_quarter(qt0, 1, xq_t[0], 0, "q0m1")
        C0(1)
        proj_q_quarter(qt0, 1, xq_t[0], 1, "q0m1")
        proj_k_half(256, 256, 0, xk_t[1])
        C0(2)
        proj_k_half(0, 256, 1, xk_t[0])
        C0(3)
        proj_k_half(512, 512, 0, xk_t[2])
        C0(4)
        proj_k_half(256, 256, 1, xk_t[1])
        C0(5)
        C0(6)
        proj_k_half(1024, 512, 0, xk_t[3])
        C0(7)
        C0(8)
        C0(9)
        proj_k_half(512, 512, 1, xk_t[2])
        C0(10)
        proj_k_half(1536, 512, 0, xk_t[4])
        C0(11)
        proj_v_slice(0, 0, xv_t[0], 0)
        C0(12)
        proj_v_slice(0, 0, xv_t[0], 1)
        C0(13)
        proj_k_half(1024, 512, 1, xk_t[3])
        proj_v_slice(0, 1, xv_t[0], 0)
        C0(14)
        proj_v_slice(0, 1, xv_t[0], 1)
        C0(15)

        # ---------- insert queue ----------
        queue = []  # (cost, fn)

        def pops(budget=1.6):
            spent = 0.0
            while queue and spent + queue[0][0] <= budget:
                cost, fn = queue.pop(0)
                fn()
                spent += cost

        def add(cost, fn):
            queue.append((cost, fn))

        # remaining ramp leftovers -> head of queue for the fill era
        add(0.9, lambda: proj_v_slice(1, 0, xv_t[1], 0))
        add(0.9, lambda: proj_v_slice(1, 0, xv_t[1], 1))
        add(0.9, lambda: proj_k_half(1536, 512, 1, xk_t[4]))
        add(0.9, lambda: proj_v_slice(1, 1, xv_t[1], 0))
        add(0.9, lambda: proj_v_slice(1, 1, xv_t[1], 1))

        # ---------- fill era: block (0,p1) ----------
        # items: remaining V, ctx(0,p0), den(0,p0), qt1-m0.  The den tree
        # writes pt slots 0,1,4,5 (lo) and 2,3,6..9 (hi): it must trail the
        # ctx duos that read those slots (d0,d2 before lo; d1,d3,d4 before
        # hi) -- FIFO pop order preserves this.
        add(0.5, lambda: emit_ctx_duo(0, 0, ptblk0, cop0))
        add(0.5, lambda: emit_ctx_duo(0, 1, ptblk0, cop0))
        add(0.9, lambda: proj_v_slice(2, 0, xv_t[2], 0))
        add(0.9, lambda: proj_v_slice(2, 0, xv_t[2], 1))
        add(0.5, lambda: emit_ctx_duo(0, 2, ptblk0, cop0))
        add(0.1, lambda: tree_lo(ptblk0, "lo_00"))
        add(0.9, lambda: proj_v_slice(2, 1, xv_t[2], 0))
        add(0.9, lambda: proj_v_slice(2, 1, xv_t[2], 1))
        add(0.5, lambda: emit_ctx_duo(0, 3, ptblk0, cop0))
        add(0.5, lambda: emit_ctx_duo(0, 4, ptblk0, cop0))
        add(0.1, lambda: tree_hi(ptblk0, "lo_00"))
        add(0.9, lambda: proj_v_slice(3, 0, xv_t[3], 0))
        add(0.9, lambda: proj_v_slice(3, 0, xv_t[3], 1))
        add(0.5, lambda: emit_ctx_duo(0, 5, ptblk0, cop0))
        add(0.15, lambda: den_mm(ptblk0, "rrc_00"))
        add(0.9, lambda: proj_v_slice(3, 1, xv_t[3], 0))
        add(0.9, lambda: proj_v_slice(3, 1, xv_t[3], 1))
        add(0.5, lambda: emit_ctx_duo(0, 6, ptblk0, cop0))
        add(0.5, lambda: emit_ctx_duo(0, 7, ptblk0, cop0))
        add(0.15, lambda: den_bc(0, 0, st["rrc_00"], cop0, "00"))
        add(0.85, lambda: proj_q_quarter(qt1, 0, xq_t[1], 0, "q1m0"))
        add(0.85, lambda: proj_q_quarter(qt1, 0, xq_t[1], 1, "q1m0"))

        for kt_i in range(NKT):
            chunk(1, kt_i, qt0, ptblk1, "f")
            pops()

        # ---------- steady blocks ----------
        blocks = [(qw_i, p) for qw_i in range(NQW) for p in range(2)][2:]
        prev = (1, 0, ptblk1, cop1, list(range(8)), "01")  # p,qw,pt,cop,duos
        qt_cur = qt1
        qt_nxt = None
        for bi, (qw_i, p) in enumerate(blocks):
            blk_i = bi + 2
            last = bi == len(blocks) - 1
            key = f"{qw_i}{p}"
            ptblk = ptp.tile([128, NKT, 2 * QW], F16, tag="pt",
                             name=f"pt{blk_i}")
            cop = copp.tile([128, QW], F32, tag="cop", name=f"cop{blk_i}")
            qt_b = qt_cur
            pp_, pqw, ppt, pcop, pduos, pkey = prev
            if p == 1 and qw_i + 1 < NQW:
                qt_nxt = qtp.tile([128, 2, QW], F16, tag="qt",
                                  name=f"qt{qw_i + 1}")
            # force-drain before this block's chunks: leftover items may
            # include this block's qt producers (PE in-order: a score that
            # waits on a later-emitted PE producer would deadlock)
            pops(budget=99.0)
            for jj in range(4):
                if jj == 0:
                    for d in pduos:
                        add(0.5, lambda a=pp_, d=d, t=ppt, c=pcop:
                            emit_ctx_duo(a, d, t, c))
                    if bi == 0:
                        # qt1-m1 for the sibling block (1,p1)
                        add(0.85, lambda: proj_q_quarter(
                            qt1, 1, xq_t[1], 0, "q1m1"))
                        add(0.85, lambda: proj_q_quarter(
                            qt1, 1, xq_t[1], 1, "q1m1"))
                    if p == 1 and qw_i + 1 < NQW:
                        nq = qw_i + 1
                        add(0.85, lambda nq=nq, qn=qt_nxt: proj_q_quarter(
                            qn, 0, xq_t[nq], 0, f"q{nq}m0"))
                        add(0.85, lambda nq=nq, qn=qt_nxt: proj_q_quarter(
                            qn, 0, xq_t[nq], 1, f"q{nq}m0"))
                if jj == 1:
                    add(0.1, lambda t=ppt, k=pkey: tree_lo(t, "lo_" + k))
                    add(0.1, lambda t=ppt, k=pkey: tree_hi(t, "lo_" + k))
                    add(0.5, lambda p=p, t=ptblk, c=cop:
                        emit_ctx_duo(p, 0, t, c))
                    add(0.5, lambda p=p, t=ptblk, c=cop:
                        emit_ctx_duo(p, 1, t, c))
                    if p == 1 and qw_i + 1 < NQW:
                        nq = qw_i + 1
                        add(0.85, lambda nq=nq, qn=qt_nxt: proj_q_quarter(
                            qn, 1, xq_t[nq], 0, f"q{nq}m1"))
                        add(0.85, lambda nq=nq, qn=qt_nxt: proj_q_quarter(
                            qn, 1, xq_t[nq], 1, f"q{nq}m1"))
                if jj == 2:
                    add(0.15, lambda t=ppt, k=pkey: den_mm(t, "rrc_" + k))
                    add(0.5, lambda p=p, t=ptblk, c=cop:
                        emit_ctx_duo(p, 2, t, c))
                    add(0.5, lambda p=p, t=ptblk, c=cop:
                        emit_ctx_duo(p, 3, t, c))
                    if p == 1 and qw_i >= 1:
                        for c in range(4):
                            add(0.5, lambda w=qw_i - 1, c=c:
                                outproj_one(w, c))
                    if last:
                        add(0.1, lambda t=ptblk: tree_lo(t, "lo_l"))
                if jj == 3:
                    add(0.15, lambda a=pp_, w=pqw, c=pcop, k=pkey:
                        den_bc(a, w, st["rrc_" + k], c, k))
                    add(0.5, lambda p=p, t=ptblk, c=cop:
                        emit_ctx_duo(p, 4, t, c))
                    add(0.5, lambda p=p, t=ptblk, c=cop:
                        emit_ctx_duo(p, 5, t, c))
                    if p == 1 and qw_i >= 1:
                        for c in range(4, 8):
                            add(0.5, lambda w=qw_i - 1, c=c:
                                outproj_one(w, c))
                    if last:
                        def last_partials(t=ptblk):
                            pl = lambda a, b: t[:, a:b, :]
                            stl = st["lo_l"]
                            nc.vector.tensor_add(pl(2, 4), pl(8, 10),
                                                 pl(10, 12))       # C: 8..11
                            nc.vector.tensor_add(pl(0, 2), pl(4, 6),
                                                 pl(2, 4))         # E+C
                            nc.vector.tensor_add(
                                stl[:, 0, :], t[:, 0, :],
                                t[:, 1, :])                        # kt0..11
                            nc.vector.tensor_add(
                                stl[:, 1, :], t[:, 12, :],
                                t[:, 13, :])                       # 12+13
                            nc.vector.tensor_add(
                                t[:, 0, :], stl[:, 0, :],
                                stl[:, 1, :])                      # S: 0..13
                            nc.vector.tensor_add(
                                t[:, 1, :], t[:, 0, :],
                                t[:, 14, :])                       # S': 0..14
                        add(0.1, last_partials)
                for half in range(2):
                    for kt_i in (4 * jj + 2 * half, 4 * jj + 2 * half + 1):
                        chunk(p, kt_i, qt_b, ptblk, f"s{blk_i}")
                    pops()
            prev = (p, qw_i, ptblk, cop, [6, 7], key)
            if p == 1 and qw_i + 1 < NQW:
                qt_cur = qt_nxt

        # ---------- tail ----------
        p_l, qw_l, ptblk_l, cop_l = prev[0], prev[1], prev[2], prev[3]
        dm = scp.tile([128, 2 * QW], F32, tag="sc", name="dm_warm")
        warm_mm = lambda: nc.tensor.matmul(
            dm[0:1, 0:64], onesK[:, 0:1], wmt[:, :], start=True, stop=True)
        # drain whatever is left (own duos 4,5 / po(w2) tail / den partials)
        emit_ctx_duo(p_l, 6, ptblk_l, cop_l)
        pops(budget=99.0)
        warm_mm()
        nc.vector.tensor_add(ptblk_l[:, 2, :], ptblk_l[:, 1, :],
                             ptblk_l[:, 15, :])          # acc = S' + kt15
        emit_ctx(p_l, 14, ptblk_l, cop_l)
        emit_ctx(p_l, 15, ptblk_l, cop_l)
        den_mm(ptblk_l, "rrc_l")
        warm_mm()
        rrc_l = st["rrc_l"]
        bc = pp.tile([128, QW], F32, tag="pp", name="bc_l")
        nc.tensor.matmul(bc[0:64, :], ones1[0:1, 0:64], rrc_l[0:1, 0:QW],
                         start=True, stop=True, tile_position=(0, 0))
        nc.tensor.matmul(bc[64:128, :], ones1[0:1, 0:64],
                         rrc_l[0:1, QW:2 * QW],
                         start=True, stop=True, tile_position=(0, 64))
        bcs = rrp.tile([128, QW], F32, tag="bcs", name="bcs_l")
        nc.vector.tensor_copy(bcs[:], bc[:])
        for tt in range(4):
            c0, c1 = tt * 128, (tt + 1) * 128
            nc.vector.tensor_mul(
                ctxP[p_l][:, qw_l * QW + c0:qw_l * QW + c1],
                cop_l[:, c0:c1], bcs[:, c0:c1])
            outproj_one(NQW - 1, 2 * tt, evq=nc.vector, dmaq=nc.sync)
            outproj_one(NQW - 1, 2 * tt + 1, evq=nc.scalar, dmaq=nc.scalar)

    nc.compile()
    return nc


def kernel(query, key, value, Wq, Wk, Wv, Wo):
    global _PROG, _LAST_IN_MAPS
    from concourse.bass_utils import run_bass_kernel_spmd

    if _PROG is None:
        _PROG = _build()
    nc = _PROG

    q2 = np.asarray(query, dtype=np.float32).reshape(B, S, D)
    k2 = np.asarray(key, dtype=np.float32).reshape(B, S, D)
    v2 = np.asarray(value, dtype=np.float32).reshape(B, S, D)
    Wq = np.asarray(Wq, dtype=np.float32)
    Wk = np.asarray(Wk, dtype=np.float32)
    Wv = np.asarray(Wv, dtype=np.float32)
    Wo = np.asarray(Wo, dtype=np.float32)

    xT = {}
    for b in range(B):
        xT[("q", b)] = np.ascontiguousarray(q2[b].T).astype(np.float16)
        xT[("k", b)] = np.ascontiguousarray(k2[b].T).astype(np.float16)
        xT[("v", b)] = np.ascontiguousarray(v2[b].T).astype(np.float16)

    in_maps = []
    for c in range(NCORES):
        b = c // 4
        l = c % 4
        rs = slice(CHD * l, CHD * (l + 1))
        in_maps.append({
            "xqT": xT[("q", b)],
            "xkT": xT[("k", b)],
            "xvT": xT[("v", b)],
            "wqT": (Wq[rs, :].T * SCALE).astype(np.float16),
            "wkT": Wk[rs, :].T.astype(np.float16),
            "wvT": Wv[rs, :].T.astype(np.float16),
            "woTs": np.ascontiguousarray(Wo[:, rs].T).astype(np.float16),
        })

    _LAST_IN_MAPS = in_maps
    res = run_bass_kernel_spmd(nc, in_maps, core_ids=list(range(NCORES)))
    parts = [res.results[c]["pout"].astype(np.float32) for c in range(NCORES)]
    out = np.empty((B, S, D), dtype=np.float32)
    for b in range(B):
        out[b] = parts[4 * b] + parts[4 * b + 1] + parts[4 * b + 2] + parts[4 * b + 3]
    return out


# revision 32
# speedup vs baseline: 1.4047x; 1.0182x over previous
"""Trainium2 Bass kernel for nn_MultiHeadAttention (B=2, S=2048, D=1024, H=16).

Sharding: 8 cores = 2 batch groups x 4 cores. Core c handles batch c//4 and
heads 4*(c%4) .. 4*(c%4)+4 (two head-pairs p=0,1). Each core computes Q/K/V
projections for its batch+heads, transposed-layout attention, and a partial
output projection over its 256 head-dims. Host sums the 4 partials per batch.

v3 design (vs v2 baseline @211.7us):
- ACT exp is the roofline (~143us busy): the schedule aims to start the exp
  stream ASAP and never starve it.
- 3 parallel DMA queues (sync/scalar HWDGE + gpsimd SWDGE); the ramp's
  critical Q/K loads split across queues (xq0 in two half-DMAs).
- all non-score PE work (projections, ctx, den matmuls, out-proj) is sliced
  into <~1us items in a FIFO insert queue, popped between score-chunk pairs
  with a per-point cost budget: the 2-deep sc PSUM ring only buffers 2.2us
  of ACT runway, so no insert slab may exceed it.
- den/bc broadcast matmuls in fp16 (were fp32 LOW_HIGH, 3x slower); the
  den->recip->bc chain is split across points so PE never waits on DVE.
- den tree adds C,D on the otherwise-idle gpsimd (DVE was co-bottleneck).
- tail: den pre-summed to one post-exp add, evictions split DVE/ACT,
  out-DMA split over both HWDGE queues, PE kept warm.
"""

import numpy as np

B, S, D, H = 2, 2048, 1024, 16
HD = D // H          # 64
NCORES = 8
HPC = 4              # heads per core
CHD = HPC * HD       # 256 head-dims per core
TOK = S              # tokens per core (one batch)
QW = 512             # query window
NQW = TOK // QW      # 4 windows
NKT = TOK // 128     # 16 key tiles
SCALE = 1.0 / np.sqrt(np.float32(D))  # 1/32, folded into Wq on host

_PROG = None
_LAST_IN_MAPS = None


def _build():
    from contextlib import ExitStack

    import concourse.bass as bass
    import concourse.tile as tile
    from concourse import bacc, mybir

    F16 = mybir.dt.float16
    F32 = mybir.dt.float32
    EXP = mybir.ActivationFunctionType.Exp

    nc = bacc.Bacc("TRN2", target_bir_lowering=False, debug=False,
                   num_devices=NCORES)

    # host-side pre-swizzled layouts: per partition, each load block's 8
    # ko-rows are contiguous (8KB descriptors -> near line-rate DMA; the
    # naive (ko ki) rearrange generates 1KB descriptors and runs
    # descriptor-rate-bound at ~60-110 GB/s/queue)
    xqT = nc.dram_tensor("xqT", [128, 8 * TOK], F16, kind="ExternalInput").ap()
    xkT = nc.dram_tensor("xkT", [128, 8 * TOK], F16, kind="ExternalInput").ap()
    xvT = nc.dram_tensor("xvT", [128, 8 * TOK], F16, kind="ExternalInput").ap()
    wqT = nc.dram_tensor("wqT", [128, 8 * CHD], F16, kind="ExternalInput").ap()
    wkT = nc.dram_tensor("wkT", [128, 8 * CHD], F16, kind="ExternalInput").ap()
    wvT = nc.dram_tensor("wvT", [128, 8 * CHD], F16, kind="ExternalInput").ap()
    woTs = nc.dram_tensor("woTs", [CHD, D], F16, kind="ExternalInput").ap()
    pout = nc.dram_tensor("pout", [TOK, D], F16, kind="ExternalOutput").ap()

    CB = 512            # V column-block width (tokens)

    with tile.TileContext(nc) as tc, ExitStack() as ctx:
        const = ctx.enter_context(tc.tile_pool(name="const", bufs=1))
        wq_sb = const.tile([128, 8, CHD], F16, tag="wq")
        wk_sb = const.tile([128, 8, CHD], F16, tag="wk")
        wv_sb = const.tile([128, 8, CHD], F16, tag="wv")
        wo_sb = [const.tile([128, D], F16, tag=f"wo{p}", name=f"wo{p}")
                 for p in range(2)]
        onesK = const.tile([128, 1], F16, tag="onesK")
        ones1 = const.tile([1, 128], F32, tag="ones1")

        nc.vector.memset(onesK, 1.0)
        nc.vector.memset(ones1, 1.0)

        warm = const.tile([1, 8], F32, tag="warm")
        nc.vector.memset(warm, 0.0)
        nc.scalar.activation(out=warm, in_=warm, func=EXP)
        wmt = const.tile([128, 64], F16, tag="wmt")
        nc.vector.memset(wmt, 1.0)

        big = ctx.enter_context(tc.tile_pool(name="big", bufs=1))
        KT = big.tile([128, 2, TOK], F16, tag="kt")          # [hd, pair, keys]
        vnat = big.tile([128, NKT, CHD], F16, tag="vnat")    # [keys, kt, hd]
        ctxP = [big.tile([128, TOK], F16, tag=f"ctxP{p}", name=f"ctxP{p}")
                for p in range(2)]

        xkb = ctx.enter_context(tc.tile_pool(name="xkb", bufs=3))
        xkb2 = ctx.enter_context(tc.tile_pool(name="xkb2", bufs=2))
        xvb = ctx.enter_context(tc.tile_pool(name="xvb", bufs=3))
        xqb = ctx.enter_context(tc.tile_pool(name="xqb", bufs=2))
        qtp = ctx.enter_context(tc.tile_pool(name="qtp", bufs=2))
        ptp = ctx.enter_context(tc.tile_pool(name="ptp", bufs=2))
        dtp = ctx.enter_context(tc.tile_pool(name="dtp", bufs=2))
        rrp = ctx.enter_context(tc.tile_pool(name="rrp", bufs=2))
        oev = ctx.enter_context(tc.tile_pool(name="oev", bufs=4))

        # PSUM (8 banks): sc ring 2x[128,1024] (4) + cop ring 2x[128,512] (2)
        # + shared proj/out/den/bcast pool 2x[128,512] (2)
        scp = ctx.enter_context(tc.tile_pool(name="scp", bufs=2, space="PSUM"))
        copp = ctx.enter_context(tc.tile_pool(name="copp", bufs=2, space="PSUM"))
        pp = ctx.enter_context(tc.tile_pool(name="pp", bufs=2, space="PSUM"))

        st = {}  # cross-item state (psum tiles, tree accumulators, rrc)

        # ---------- emission helpers ----------
        def dma_x_block(pool, src, off, w, name, queue=None, halves=False,
                        gate=None):
            # off = per-partition element offset of this block in the
            # swizzled source; the block's [8, w] region is contiguous.
            t = pool.tile([128, 8, w], F16, tag="xb", name=name)
            r = src[:, off:off + 8 * w].rearrange("ki (ko t) -> ki ko t",
                                                  ko=8)
            if gate is not None:
                # Tile's scheduler hoists ready DMAs into idle DGE slots
                # regardless of emission order; a tiny write into the DMA
                # target (reading a late tile) gives the DMA a real WAW
                # dependency so non-critical loads don't steal HBM bandwidth
                # from the ramp-critical ones.
                nc.gpsimd.tensor_copy(t[:, 0, 0:8], gate)
            if halves:
                nc.sync.dma_start(out=t[:, 0:4, :], in_=r[:, 0:4, :])
                nc.scalar.dma_start(out=t[:, 4:8, :], in_=r[:, 4:8, :])
            else:
                (queue or nc.sync).dma_start(out=t, in_=r)
            return t

        def proj_q_quarter(qt, m, xq_t, phase, name):
            if phase == 0:
                st[name] = pp.tile([128, QW], F32, tag="pp", name=name)
            psq = st[name]
            for ko in range(4 * phase, 4 * phase + 4):
                nc.tensor.matmul(
                    psq[:], wq_sb[:, ko, m * 128:(m + 1) * 128],
                    xq_t[:, ko, :], start=(ko == 0), stop=(ko == 7))
            if phase == 1:
                nc.vector.tensor_copy(qt[:, m, :], psq[:])
                del st[name]

        def proj_k_half(c0, w, m, xk_t):
            psk = pp.tile([128, w], F32, tag="pp", name=f"psk{c0}_{m}")
            for ko in range(8):
                nc.tensor.matmul(
                    psk[:], wk_sb[:, ko, m * 128:(m + 1) * 128],
                    xk_t[:, ko, :], start=(ko == 0), stop=(ko == 7))
            nc.vector.tensor_copy(KT[:, m, c0:c0 + w], psk[:])

        def proj_v_slice(blk, half, xv_t, tt):
            name = f"pv{blk}_{half}"
            if tt == 0:
                st[name] = pp.tile([128, 2 * CHD], F32, tag="pp", name=name)
            pv = st[name]
            t0 = (2 * half + tt) * 128
            for ko in range(8):
                nc.tensor.matmul(
                    pv[:, tt * CHD:(tt + 1) * CHD],
                    xv_t[:, ko, t0:t0 + 128],
                    wv_sb[:, ko, :], start=(ko == 0), stop=(ko == 7))
            if tt == 1:
                nc.vector.tensor_copy(
                    vnat[:, 4 * blk + 2 * half:4 * blk + 2 * half + 2, :],
                    bass.AP(tensor=pv.tensor, offset=pv.offset,
                            ap=[list(pv.ap[0]), [CHD, 2], [1, CHD]]))
                del st[name]

        def chunk(p, kt_i, qt, ptblk, tag=""):
            k0 = kt_i * 128
            sc = scp.tile([128, 2 * QW], F32, tag="sc",
                          name=f"sc{tag}_{kt_i}")
            nc.tensor.matmul(
                sc[:, 0:QW], KT[0:64, p, k0:k0 + 128], qt[0:64, p, :],
                start=True, stop=True, tile_position=(0, 0))
            nc.tensor.matmul(
                sc[:, QW:2 * QW], KT[64:128, p, k0:k0 + 128], qt[64:128, p, :],
                start=True, stop=True, tile_position=(64, 0))
            nc.scalar.activation(out=ptblk[:, kt_i, :], in_=sc[:], func=EXP)

        def emit_ctx(p, kt_i, ptblk, cop):
            h0 = p * 128
            nc.tensor.matmul(
                cop[0:64, :], vnat[:, kt_i, h0:h0 + 64],
                ptblk[:, kt_i, 0:QW],
                start=(kt_i == 0), stop=(kt_i == NKT - 1),
                tile_position=(0, 0))
            nc.tensor.matmul(
                cop[64:128, :], vnat[:, kt_i, h0 + 64:h0 + 128],
                ptblk[:, kt_i, QW:2 * QW],
                start=(kt_i == 0), stop=(kt_i == NKT - 1),
                tile_position=(0, 64))

        def emit_ctx_duo(p, d, ptblk, cop):
            emit_ctx(p, 2 * d, ptblk, cop)
            emit_ctx(p, 2 * d + 1, ptblk, cop)

        def tree_lo(ptblk, key):
            stt = dtp.tile([128, 2, 2 * QW], F16, tag="dt", name=f"dt_{key}")
            pv = lambda a, b: ptblk[:, a:b, :]
            nc.vector.tensor_add(stt[:, 0:2, :], pv(0, 2), pv(2, 4))    # A
            nc.vector.tensor_add(pv(0, 2), pv(4, 6), pv(6, 8))          # B
            nc.vector.tensor_add(pv(4, 6), stt[:, 0:2, :], pv(0, 2))    # E
            st[key] = stt

        def tree_hi(ptblk, key):
            # all-DVE: gpsimd adds are ~4x the latency and sit on the
            # ptp-ring critical path (exp of block i+2 waits on them)
            pv = lambda a, b: ptblk[:, a:b, :]
            nc.vector.tensor_add(pv(2, 4), pv(8, 10), pv(10, 12))       # C
            nc.vector.tensor_add(pv(6, 8), pv(12, 14), pv(14, 16))      # D
            nc.vector.tensor_add(pv(8, 10), pv(2, 4), pv(6, 8))         # F
            nc.vector.tensor_add(pv(0, 2), pv(4, 6), pv(8, 10))         # G
            nc.vector.tensor_add(ptblk[:, 2, :], ptblk[:, 0, :],
                                 ptblk[:, 1, :])                        # acc

        def den_mm(ptblk, key):
            """ones-matmul partition reduce per head, reciprocal into rrc."""
            accv = ptblk[:, 2, :]
            rrc = rrp.tile([1, 2 * QW], F32, tag="rrc", name=f"rrc_{key}")
            for h in range(2):
                den = pp.tile([128, QW], F32, tag="pp", name=f"den_{key}_{h}")
                nc.tensor.matmul(den[0:1, :], onesK[:, 0:1],
                                 accv[:, h * QW:(h + 1) * QW],
                                 start=True, stop=True)
                nc.vector.reciprocal_approx_fast(
                    rrc[0:1, h * QW:(h + 1) * QW], den[0:1, :])
            st[key] = rrc

        def den_bc(p, qw_i, rrc, cop, key):
            bc = pp.tile([128, QW], F32, tag="pp", name=f"bc_{key}")
            nc.tensor.matmul(bc[0:64, :], ones1[0:1, 0:64], rrc[0:1, 0:QW],
                             start=True, stop=True, tile_position=(0, 0))
            nc.tensor.matmul(bc[64:128, :], ones1[0:1, 0:64],
                             rrc[0:1, QW:2 * QW],
                             start=True, stop=True, tile_position=(0, 64))
            bcs = rrp.tile([128, QW], F32, tag="bcs", name=f"bcs_{key}")
            nc.vector.tensor_copy(bcs[:], bc[:])
            nc.vector.tensor_mul(
                ctxP[p][:, qw_i * QW:(qw_i + 1) * QW], cop[:], bcs[:])

        def outproj_one(qw_i, c, evq=None, dmaq=None):
            tt, et = c // 2, c % 2
            t0 = qw_i * QW + tt * 128
            po = pp.tile([128, 512], F32, tag="pp",
                         name=f"po{qw_i}_{tt}_{et}")
            for p in range(2):
                nc.tensor.matmul(
                    po[:], ctxP[p][:, t0:t0 + 128],
                    wo_sb[p][:, et * 512:(et + 1) * 512],
                    start=(p == 0), stop=(p == 1))
            ev = oev.tile([128, 512], F16, tag="oev")
            if evq is nc.scalar:
                nc.scalar.copy(ev[:], po[:])
            else:
                nc.vector.tensor_copy(ev[:], po[:])
            (dmaq or nc.sync).dma_start(
                out=pout[t0:t0 + 128, et * 512:(et + 1) * 512],
                in_=ev[:])

        # ---------- DMA issue ----------
        # sync:   xq0a, wk, xk[512:1024], xk[1536:2048], out-DMA w0..w2
        # scalar: xq0b, xk[0:256], xk[256:512], xk[1024:1536], xq1..3
        # gpsimd: wq, wv, xv0..2, wo0, wo1, xv3
        xq_t = [None] * NQW
        xq_t[0] = dma_x_block(xqb, xqT, 0, QW, "xq0", halves=True)
        nc.gpsimd.dma_start(
            out=wq_sb, in_=wqT.rearrange("ki (ko m) -> ki ko m", ko=8))
        # scalar (= the exp engine's queue) carries ONLY pre-stream loads:
        # a DMA_DIRECT2D's DGE time (1-13us under ring backpressure) blocks
        # every ACTIVATE behind it in the queue.
        KW = [256, 256, 512, 512, 512]
        KC0 = [0, 256, 512, 1024, 1536]
        KOFF = [0, 2048, 4096, 8192, 12288]
        KQ = [nc.scalar, nc.scalar, nc.sync, None, nc.sync]
        nc.sync.dma_start(
            out=wk_sb, in_=wkT.rearrange("ki (ko m) -> ki ko m", ko=8))
        xk_t = [dma_x_block(xkb if w == 512 else xkb2, xkT, o, w,
                            f"xk{c0}", queue=q) if q is not None else None
                for c0, o, w, q in zip(KC0, KOFF, KW, KQ)]

        # hold the PE clock gate open through the ramp's DMA shadow
        dmw = copp.tile([128, QW], F32, tag="cop", name="dm_ramp")
        def pe_warm(n):
            for _ in range(n):
                nc.tensor.matmul(dmw[0:1, 0:64], onesK[:, 0:1], wmt[:, :],
                                 start=True, stop=True)
        pe_warm(70)

        qt0 = qtp.tile([128, 2, QW], F16, tag="qt", name="qt0")
        qt1 = qtp.tile([128, 2, QW], F16, tag="qt", name="qt1")
        ptblk0 = ptp.tile([128, NKT, 2 * QW], F16, tag="pt", name="pt0")
        ptblk1 = ptp.tile([128, NKT, 2 * QW], F16, tag="pt", name="pt1")
        cop0 = copp.tile([128, QW], F32, tag="cop", name="cop0")
        cop1 = copp.tile([128, QW], F32, tag="cop", name="cop1")

        # ---------- ramp: block (0,p0), explicit schedule ----------
        C0 = lambda kt: chunk(0, kt, qt0, ptblk0, "r")
        proj_q_quarter(qt0, 0, xq_t[0], 0, "q0m0")
        proj_q_quarter(qt0, 0, xq_t[0], 1, "q0m0")
        # gpsimd-queue guard: V/wo loads (4.5MB) must not steal HBM
        # bandwidth from the ramp-critical Q/K loads (SDMA round-robins
        # across queues).  The tiny copy below waits for the qt0-m0
        # eviction, stalling the gpsimd DGE until the critical window ends.
        # gpsimd loads ordered by need-time, all gated on the qt0-m0 evict
        # (K feeds ACT directly; V/xq lateness is absorbed by the queue)
        gt = qt0[:, 0, 0:8]
        xk_t[3] = dma_x_block(xkb, xkT, KOFF[3], KW[3], "xk1024",
                              queue=nc.gpsimd, gate=gt)
        nc.gpsimd.tensor_copy(wv_sb[:, 0, 0:8], gt)
        nc.gpsimd.dma_start(
            out=wv_sb, in_=wvT.rearrange("ki (ko m) -> ki ko m", ko=8))
        xv_t = [dma_x_block(xvb, xvT, b * 8 * CB, CB, f"xv{b}",
                            queue=nc.gpsimd, gate=gt)
                for b in range(2)]
        xq_t[1] = dma_x_block(xqb, xqT, 8 * QW, QW, "xq1", queue=nc.gpsimd,
                              gate=gt)
        xv_t.append(dma_x_block(xvb, xvT, 2 * 8 * CB, CB, "xv2",
                                queue=nc.gpsimd, gate=gt))
        xv_t.append(dma_x_block(xvb, xvT, 3 * 8 * CB, CB, "xv3",
                                queue=nc.gpsimd, gate=gt))
        nc.gpsimd.tensor_copy(wo_sb[0][:, 0:8], gt)
        nc.gpsimd.dma_start(out=wo_sb[0], in_=woTs[0:128, :])
        nc.gpsimd.tensor_copy(wo_sb[1][:, 0:8], gt)
        nc.gpsimd.dma_start(out=wo_sb[1], in_=woTs[128:256, :])
        xq_t[2] = dma_x_block(xqb, xqT, 2 * 8 * QW, QW, "xq2",
                              queue=nc.gpsimd, gate=gt)
        xq_t[3] = dma_x_block(xqb, xqT, 3 * 8 * QW, QW, "xq3",
                              queue=nc.gpsimd, gate=gt)
        pe_warm(25)
        proj_k_half(0, 256, 0, xk_t[0])
        C0(0)
        proj_q_quarter(qt0, 1, xq_t[0], 0, "q0m1")
        C0(1)
        proj_q_quarter(qt0, 1, xq_t[0], 1, "q0m1")
        proj_k_half(256, 256, 0, xk_t[1])
        C0(2)
        proj_k_half(0, 256, 1, xk_t[0])
        C0(3)
        proj_k_half(512, 512, 0, xk_t[2])
        C0(4)
        proj_k_half(256, 256, 1, xk_t[1])
        C0(5)
        C0(6)
        proj_k_half(1024, 512, 0, xk_t[3])
        C0(7)
        C0(8)
        C0(9)
        proj_k_half(512, 512, 1, xk_t[2])
        C0(10)
        proj_k_half(1536, 512, 0, xk_t[4])
        C0(11)
        proj_v_slice(0, 0, xv_t[0], 0)
        C0(12)
        proj_v_slice(0, 0, xv_t[0], 1)
        C0(13)
        proj_k_half(1024, 512, 1, xk_t[3])
        proj_v_slice(0, 1, xv_t[0], 0)
        C0(14)
        proj_v_slice(0, 1, xv_t[0], 1)
        C0(15)

        # ---------- insert queue ----------
        queue = []  # (cost, fn)

        def pops(budget=1.6):
            spent = 0.0
            while queue and spent + queue[0][0] <= budget:
                cost, fn = queue.pop(0)
                fn()
                spent += cost

        def add(cost, fn):
            queue.append((cost, fn))

        # remaining ramp leftovers -> head of queue for the fill era
        add(0.9, lambda: proj_v_slice(1, 0, xv_t[1], 0))
        add(0.9, lambda: proj_v_slice(1, 0, xv_t[1], 1))
        add(0.9, lambda: proj_k_half(1536, 512, 1, xk_t[4]))
        add(0.9, lambda: proj_v_slice(1, 1, xv_t[1], 0))
        add(0.9, lambda: proj_v_slice(1, 1, xv_t[1], 1))

        # ---------- fill era: block (0,p1) ----------
        # items: remaining V, ctx(0,p0), den(0,p0), qt1-m0.  The den tree
        # writes pt slots 0,1,4,5 (lo) and 2,3,6..9 (hi): it must trail the
        # ctx duos that read those slots (d0,d2 before lo; d1,d3,d4 before
        # hi) -- FIFO pop order preserves this.
        add(0.5, lambda: emit_ctx_duo(0, 0, ptblk0, cop0))
        add(0.5, lambda: emit_ctx_duo(0, 1, ptblk0, cop0))
        add(0.9, lambda: proj_v_slice(2, 0, xv_t[2], 0))
        add(0.9, lambda: proj_v_slice(2, 0, xv_t[2], 1))
        add(0.5, lambda: emit_ctx_duo(0, 2, ptblk0, cop0))
        add(0.1, lambda: tree_lo(ptblk0, "lo_00"))
        add(0.9, lambda: proj_v_slice(2, 1, xv_t[2], 0))
        add(0.9, lambda: proj_v_slice(2, 1, xv_t[2], 1))
        add(0.5, lambda: emit_ctx_duo(0, 3, ptblk0, cop0))
        add(0.5, lambda: emit_ctx_duo(0, 4, ptblk0, cop0))
        add(0.1, lambda: tree_hi(ptblk0, "lo_00"))
        add(0.9, lambda: proj_v_slice(3, 0, xv_t[3], 0))
        add(0.9, lambda: proj_v_slice(3, 0, xv_t[3], 1))
        add(0.5, lambda: emit_ctx_duo(0, 5, ptblk0, cop0))
        add(0.15, lambda: den_mm(ptblk0, "rrc_00"))
        add(0.9, lambda: proj_v_slice(3, 1, xv_t[3], 0))
        add(0.9, lambda: proj_v_slice(3, 1, xv_t[3], 1))
        add(0.5, lambda: emit_ctx_duo(0, 6, ptblk0, cop0))
        add(0.5, lambda: emit_ctx_duo(0, 7, ptblk0, cop0))
        add(0.15, lambda: den_bc(0, 0, st["rrc_00"], cop0, "00"))
        add(0.85, lambda: proj_q_quarter(qt1, 0, xq_t[1], 0, "q1m0"))
        add(0.85, lambda: proj_q_quarter(qt1, 0, xq_t[1], 1, "q1m0"))

        for kt_i in range(NKT):
            chunk(1, kt_i, qt0, ptblk1, "f")
            pops()

        # ---------- steady blocks ----------
        blocks = [(qw_i, p) for qw_i in range(NQW) for p in range(2)][2:]
        prev = (1, 0, ptblk1, cop1, list(range(8)), "01")  # p,qw,pt,cop,duos
        qt_cur = qt1
        qt_nxt = None
        for bi, (qw_i, p) in enumerate(blocks):
            blk_i = bi + 2
            last = bi == len(blocks) - 1
            key = f"{qw_i}{p}"
            ptblk = ptp.tile([128, NKT, 2 * QW], F16, tag="pt",
                             name=f"pt{blk_i}")
            cop = copp.tile([128, QW], F32, tag="cop", name=f"cop{blk_i}")
            qt_b = qt_cur
            pp_, pqw, ppt, pcop, pduos, pkey = prev
            if p == 1 and qw_i + 1 < NQW:
                qt_nxt = qtp.tile([128, 2, QW], F16, tag="qt",
                                  name=f"qt{qw_i + 1}")
            # force-drain before this block's chunks: leftover items may
            # include this block's qt producers (PE in-order: a score that
            # waits on a later-emitted PE producer would deadlock)
            pops(budget=99.0)
            for jj in range(4):
                if jj == 0:
                    for d in pduos:
                        add(0.5, lambda a=pp_, d=d, t=ppt, c=pcop:
                            emit_ctx_duo(a, d, t, c))
                    if bi == 0:
                        # qt1-m1 for the sibling block (1,p1)
                        add(0.85, lambda: proj_q_quarter(
                            qt1, 1, xq_t[1], 0, "q1m1"))
                        add(0.85, lambda: proj_q_quarter(
                            qt1, 1, xq_t[1], 1, "q1m1"))
                    if p == 1 and qw_i + 1 < NQW:
                        nq = qw_i + 1
                        add(0.85, lambda nq=nq, qn=qt_nxt: proj_q_quarter(
                            qn, 0, xq_t[nq], 0, f"q{nq}m0"))
                        add(0.85, lambda nq=nq, qn=qt_nxt: proj_q_quarter(
                            qn, 0, xq_t[nq], 1, f"q{nq}m0"))
                if jj == 1:
                    add(0.1, lambda t=ppt, k=pkey: tree_lo(t, "lo_" + k))
                    add(0.1, lambda t=ppt, k=pkey: tree_hi(t, "lo_" + k))
                    add(0.5, lambda p=p, t=ptblk, c=cop:
                        emit_ctx_duo(p, 0, t, c))
                    add(0.5, lambda p=p, t=ptblk, c=cop:
                        emit_ctx_duo(p, 1, t, c))
                    if p == 1 and qw_i + 1 < NQW:
                        nq = qw_i + 1
                        add(0.85, lambda nq=nq, qn=qt_nxt: proj_q_quarter(
                            qn, 1, xq_t[nq], 0, f"q{nq}m1"))
                        add(0.85, lambda nq=nq, qn=qt_nxt: proj_q_quarter(
                            qn, 1, xq_t[nq], 1, f"q{nq}m1"))
                if jj == 2:
                    add(0.15, lambda t=ppt, k=pkey: den_mm(t, "rrc_" + k))
                    add(0.5, lambda p=p, t=ptblk, c=cop:
                        emit_ctx_duo(p, 2, t, c))
                    add(0.5, lambda p=p, t=ptblk, c=cop:
                        emit_ctx_duo(p, 3, t, c))
                    if p == 1 and qw_i >= 1:
                        for c in range(4):
                            add(0.5, lambda w=qw_i - 1, c=c:
                                outproj_one(w, c))
                    if last:
                        add(0.1, lambda t=ptblk: tree_lo(t, "lo_l"))
                if jj == 3:
                    add(0.15, lambda a=pp_, w=pqw, c=pcop, k=pkey:
                        den_bc(a, w, st["rrc_" + k], c, k))
                    add(0.5, lambda p=p, t=ptblk, c=cop:
                        emit_ctx_duo(p, 4, t, c))
                    add(0.5, lambda p=p, t=ptblk, c=cop:
                        emit_ctx_duo(p, 5, t, c))
                    if last:
                        # part 1 needs only exps <= kt11: emit before the
                        # po(w2) evicts in the DVE FIFO
                        def lp1(t=ptblk):
                            pl = lambda a, b: t[:, a:b, :]
                            stl = st["lo_l"]
                            nc.vector.tensor_add(pl(2, 4), pl(8, 10),
                                                 pl(10, 12))       # C: 8..11
                            nc.vector.tensor_add(pl(0, 2), pl(4, 6),
                                                 pl(2, 4))         # E+C
                            nc.vector.tensor_add(
                                stl[:, 0, :], t[:, 0, :],
                                t[:, 1, :])                        # kt0..11
                        add(0.1, lp1)
                    if p == 1 and qw_i >= 1:
                        for c in range(4, 8):
                            add(0.5, lambda w=qw_i - 1, c=c:
                                outproj_one(w, c))
                    if last:
                        def lp2(t=ptblk):
                            stl = st["lo_l"]
                            nc.vector.tensor_add(
                                stl[:, 1, :], t[:, 12, :],
                                t[:, 13, :])                       # 12+13
                            nc.vector.tensor_add(
                                t[:, 0, :], stl[:, 0, :],
                                stl[:, 1, :])                      # S: 0..13
                            nc.vector.tensor_add(
                                t[:, 1, :], t[:, 0, :],
                                t[:, 14, :])                       # S': 0..14
                        add(0.1, lp2)
                for half in range(2):
                    for kt_i in (4 * jj + 2 * half, 4 * jj + 2 * half + 1):
                        chunk(p, kt_i, qt_b, ptblk, f"s{blk_i}")
                    pops()
            prev = (p, qw_i, ptblk, cop, [6, 7], key)
            if p == 1 and qw_i + 1 < NQW:
                qt_cur = qt_nxt

        # ---------- tail ----------
        p_l, qw_l, ptblk_l, cop_l = prev[0], prev[1], prev[2], prev[3]
        dm = scp.tile([128, 2 * QW], F32, tag="sc", name="dm_warm")
        warm_mm = lambda: nc.tensor.matmul(
            dm[0:1, 0:64], onesK[:, 0:1], wmt[:, :], start=True, stop=True)
        # drain whatever is left (own duos 4,5 / po(w2) tail / den partials)
        emit_ctx_duo(p_l, 6, ptblk_l, cop_l)
        pops(budget=99.0)
        warm_mm()
        nc.vector.tensor_add(ptblk_l[:, 2, :], ptblk_l[:, 1, :],
                             ptblk_l[:, 15, :])          # acc = S' + kt15
        emit_ctx(p_l, 14, ptblk_l, cop_l)
        emit_ctx(p_l, 15, ptblk_l, cop_l)
        den_mm(ptblk_l, "rrc_l")
        warm_mm()
        rrc_l = st["rrc_l"]
        # per-token-tile broadcast + normalize + out-projection: the bc
        # matmuls (fp32 LOW_HIGH) are sliced N=128 so only 1/4 of the
        # broadcast sits on the critical chain; bc lives in the scp ring's
        # dm tile (the pp ring would recycle it under the po allocations)
        bcs = rrp.tile([128, QW], F32, tag="bcs", name="bcs_l")
        for tt in range(4):
            c0, c1 = tt * 128, (tt + 1) * 128
            nc.tensor.matmul(dm[0:64, c0:c1], ones1[0:1, 0:64],
                             rrc_l[0:1, c0:c1],
                             start=True, stop=True, tile_position=(0, 0))
            nc.tensor.matmul(dm[64:128, c0:c1], ones1[0:1, 0:64],
                             rrc_l[0:1, QW + c0:QW + c1],
                             start=True, stop=True, tile_position=(0, 64))
            nc.vector.tensor_copy(bcs[:, c0:c1], dm[:, c0:c1])
            nc.vector.tensor_mul(
                ctxP[p_l][:, qw_l * QW + c0:qw_l * QW + c1],
                cop_l[:, c0:c1], bcs[:, c0:c1])
            outproj_one(NQW - 1, 2 * tt, evq=nc.vector, dmaq=nc.sync)
            outproj_one(NQW - 1, 2 * tt + 1, evq=nc.scalar, dmaq=nc.scalar)

    nc.compile()
    return nc


def kernel(query, key, value, Wq, Wk, Wv, Wo):
    global _PROG, _LAST_IN_MAPS
    from concourse.bass_utils import run_bass_kernel_spmd

    if _PROG is None:
        _PROG = _build()
    nc = _PROG

    q2 = np.asarray(query, dtype=np.float32).reshape(B, S, D)
    k2 = np.asarray(key, dtype=np.float32).reshape(B, S, D)
    v2 = np.asarray(value, dtype=np.float32).reshape(B, S, D)
    Wq = np.asarray(Wq, dtype=np.float32)
    Wk = np.asarray(Wk, dtype=np.float32)
    Wv = np.asarray(Wv, dtype=np.float32)
    Wo = np.asarray(Wo, dtype=np.float32)

    # swizzle [D, cols] -> [128, sum(8*w)]: per partition, each col-block's
    # 8 ko-rows contiguous (big DMA descriptors)
    def swz(arr, blocks):
        A = arr.reshape(8, 128, arr.shape[1])
        parts = [np.ascontiguousarray(
            A[:, :, c0:c0 + w].transpose(1, 0, 2)).reshape(128, 8 * w)
            for c0, w in blocks]
        return np.ascontiguousarray(np.concatenate(parts, axis=1)).astype(
            np.float16)

    QBLK = [(i * 512, 512) for i in range(4)]
    KBLK = [(0, 256), (256, 256), (512, 512), (1024, 512), (1536, 512)]
    WBLK = [(0, CHD)]

    xT = {}
    for b in range(B):
        xT[("q", b)] = swz(q2[b].T, QBLK)
        xT[("k", b)] = swz(k2[b].T, KBLK)
        xT[("v", b)] = swz(v2[b].T, QBLK)

    in_maps = []
    for c in range(NCORES):
        b = c // 4
        l = c % 4
        rs = slice(CHD * l, CHD * (l + 1))
        in_maps.append({
            "xqT": xT[("q", b)],
            "xkT": xT[("k", b)],
            "xvT": xT[("v", b)],
            "wqT": swz(Wq[rs, :].T * SCALE, WBLK),
            "wkT": swz(Wk[rs, :].T, WBLK),
            "wvT": swz(Wv[rs, :].T, WBLK),
            "woTs": np.ascontiguousarray(Wo[:, rs].T).astype(np.float16),
        })

    _LAST_IN_MAPS = in_maps
    res = run_bass_kernel_spmd(nc, in_maps, core_ids=list(range(NCORES)))
    parts = [res.results[c]["pout"].astype(np.float32) for c in range(NCORES)]
    out = np.empty((B, S, D), dtype=np.float32)
    for b in range(B):
        out[b] = parts[4 * b] + parts[4 * b + 1] + parts[4 * b + 2] + parts[4 * b + 3]
    return out
